# revision 12
# baseline (speedup 1.0000x reference)
"""Trainium2 Bass kernel for nn_MultiHeadMALAAttention.

Sharding: 8 cores; core c handles batch b = c//2, token half h = c%2
(tokens [h*4096, (h+1)*4096) of N=8192).  Stats (ksum, vsum, kv gram)
need full-N reductions -> pairwise AllReduce between the two cores of a
batch, replica groups [[0,1],[2,3],[4,5],[6,7]].

Structure (per core), ordered so the AllReduce hides under compute:
  dummy 32B AllReduce first (absorbs the one-time CC barrier under the
  initial weight/x DMAs)
  phase A: k,v projections -> elu(k)+1 -> rope(k) -> PE transposes ->
           kv gram accumulation in PSUM; ksum/vsum ride accum_out.
  stats pack ([128,68]) -> AllReduce
  phase B: q,o projections -> elu(q)+1 -> rope(q) -> qs  (runs during AR)
  phase C: z = zblk^T q1; attn = kvblk^T qs; then everything else on DVE:
           res = (1+1/z)*attn - z*vmean + lepe(v), y = res*o, out proj.

Key fusions vs a naive pipeline: elu+1 = min(exp(x),1)+relu(x) (2 ACT +
1 stt); the (1+1/z) factor commutes past the per-head kv matmul so
rope(q) is stats-independent; -z*vmean rides the z PSUM as one stt; the
depthwise conv (LEPE) is 3 stt ops with per-partition channel weights.
"""

import os
import sys

sys.path.insert(0, "/opt/trn_rl_repo")

import numpy as np
import ml_dtypes

B, N, DIM, H, HD = 4, 8192, 256, 8, 32
INTERNAL = H * HD  # 256
SCALE = HD ** -0.5
NCORES = 8
T = N // 2          # tokens per core
TH = T + 2          # with 1-token halo each side
CH = 512            # chunk tokens
NCH = T // CH       # chunks per core
KSC = SCALE / N     # kv_state scale (s^2)

BF16 = ml_dtypes.bfloat16


# ---------------------------------------------------------------- host prep

def _host_prep(x, sin, cos, W_qkvo, b_qkvo, W_lepe, b_lepe, W_proj, b_proj):
    """Build per-core input dicts (all device tensors)."""
    assert not (np.any(b_qkvo) or np.any(b_lepe) or np.any(b_proj)), \
        "bias path not implemented (setup_inputs uses zero biases)"

    WT = W_qkvo.T.astype(np.float32)          # [DIM, 1024] = lhsT layout
    wq = WT[:, 0:256].astype(BF16)
    wkv = WT[:, 256:768].astype(BF16)          # k cols 0:256, v cols 256:512
    wo = WT[:, 768:1024].astype(BF16)
    wp = W_proj.T.astype(np.float32).astype(BF16)   # [DIM, 256] rhs layout
    wl = W_lepe[:, 0, :].astype(np.float32)    # [256, 3]
    # per-partition lepe weights: tile j cols 3j..3j+3
    wl6 = np.zeros((128, 6), np.float32)
    wl6[:, 0:3] = wl[0:128]
    wl6[:, 3:6] = wl[128:256]

    # rotate-every-two matrix as lhsT: rot = R.T @ x ; R[k, m] = coeff of
    # chan k in rot-chan m:  rot[2i] = -x[2i+1], rot[2i+1] = x[2i]
    R = np.zeros((128, 128), np.float32)
    for i in range(64):
        R[2 * i + 1, 2 * i] = -1.0
        R[2 * i, 2 * i + 1] = 1.0
    R = R.astype(BF16)

    hmask = np.zeros((128, 128), np.float32)
    for hh in range(4):
        hmask[32 * hh:32 * (hh + 1), 32 * hh:32 * (hh + 1)] = 1.0
    hmask = hmask.astype(BF16)

    ident16 = np.eye(128, dtype=np.float32).astype(BF16)

    xf = np.asarray(x, np.float32)
    sinf = np.asarray(sin, np.float32)
    cosf = np.asarray(cos, np.float32)

    per_core = []
    for c in range(NCORES):
        b = c // 2
        t0 = (c % 2) * T
        # x channel-major with halo [256, TH]
        xpad = np.zeros((TH, DIM), np.float32)
        lo, hi = t0 - 1, t0 + T + 1
        slo, shi = max(lo, 0), min(hi, N)
        xpad[slo - lo: slo - lo + (shi - slo)] = xf[b, slo:shi]
        xct = np.ascontiguousarray(xpad.T).astype(BF16)          # [256, TH]

        srep = np.tile(sinf[t0:t0 + T].T, (4, 1)).astype(BF16)   # [128, T]
        crep = np.tile(cosf[t0:t0 + T].T, (4, 1)).astype(BF16)   # [128, T]

        per_core.append({
            "xct": xct, "srep": np.ascontiguousarray(srep),
            "crep": np.ascontiguousarray(crep),
            "wq": np.ascontiguousarray(wq), "wkv": np.ascontiguousarray(wkv),
            "wo": np.ascontiguousarray(wo), "wp": np.ascontiguousarray(wp),
            "rblk": R, "hmask": hmask, "ident16": ident16, "wl6": wl6,
        })
    return per_core


# ------------------------------------------------------------ device kernel

def _build_nc():
    from concourse import bacc
    import concourse.mybir as mybir
    import concourse.tile as tile

    dt = mybir.dt
    AF = mybir.ActivationFunctionType
    OP = mybir.AluOpType

    nocc = bool(os.environ.get("KERNEL_NOCC"))

    nc = bacc.Bacc(None, target_bir_lowering=False)

    # ---- I/O
    xct_d = nc.dram_tensor("xct", [256, TH], dt.bfloat16, kind="ExternalInput")
    srep_d = nc.dram_tensor("srep", [128, T], dt.bfloat16, kind="ExternalInput")
    crep_d = nc.dram_tensor("crep", [128, T], dt.bfloat16, kind="ExternalInput")
    wq_d = nc.dram_tensor("wq", [256, 256], dt.bfloat16, kind="ExternalInput")
    wkv_d = nc.dram_tensor("wkv", [256, 512], dt.bfloat16, kind="ExternalInput")
    wo_d = nc.dram_tensor("wo", [256, 256], dt.bfloat16, kind="ExternalInput")
    wp_d = nc.dram_tensor("wp", [256, 256], dt.bfloat16, kind="ExternalInput")
    rblk_d = nc.dram_tensor("rblk", [128, 128], dt.bfloat16, kind="ExternalInput")
    hmask_d = nc.dram_tensor("hmask", [128, 128], dt.bfloat16, kind="ExternalInput")
    id16_d = nc.dram_tensor("ident16", [128, 128], dt.bfloat16, kind="ExternalInput")
    wl6_d = nc.dram_tensor("wl6", [128, 6], dt.float32, kind="ExternalInput")
    out_d = nc.dram_tensor("out", [T, 256], dt.float32, kind="ExternalOutput")
    dbg = bool(os.environ.get("KERNEL_DBG"))
    if dbg:
        dbg16_d = nc.dram_tensor("dbg16", [128, 4096], dt.bfloat16,
                                 kind="ExternalOutput")
        dbg32_d = nc.dram_tensor("dbg32", [128, 68], dt.float32,
                                 kind="ExternalOutput")

    RG = [[0, 1], [2, 3], [4, 5], [6, 7]]

    with tile.TileContext(nc) as tc:
        with (
            tc.tile_pool(name="const", bufs=1) as const,
            tc.tile_pool(name="work", bufs=2) as work,
            tc.tile_pool(name="psum", bufs=1, space="PSUM") as ppool,
            tc.tile_pool(name="dram", bufs=1, space="DRAM") as dpool,
        ):
            # ---- dummy collective: absorb the one-time CC barrier early
            if not nocc:
                dum = const.tile([1, 8], dt.float32, tag="dum", name="dum")
                nc.vector.memset(dum, 0.0)
                ccd_i = dpool.tile([1, 8], dt.float32, tag="ccdi", name="ccdi")
                ccd_o = dpool.tile([1, 8], dt.float32, tag="ccdo", name="ccdo")
                nc.gpsimd.dma_start(out=ccd_i[:, :], in_=dum)
                nc.gpsimd.collective_compute(
                    "AllReduce", OP.add, replica_groups=RG,
                    ins=[ccd_i[:, :]], outs=[ccd_o[:, :]])

            # ---- consts
            def load(tname, dten, shape, dtype=dt.bfloat16):
                t_ = const.tile(shape, dtype, tag=tname, name=tname)
                nc.sync.dma_start(out=t_, in_=dten[:, :])
                return t_

            wq = [const.tile([128, 256], dt.bfloat16, tag=f"wq{k}", name=f"wq{k}")
                  for k in range(2)]
            wkv = [const.tile([128, 512], dt.bfloat16, tag=f"wkv{k}", name=f"wkv{k}")
                   for k in range(2)]
            wo = [const.tile([128, 256], dt.bfloat16, tag=f"wo{k}", name=f"wo{k}")
                  for k in range(2)]
            wp = [const.tile([128, 256], dt.bfloat16, tag=f"wp{k}", name=f"wp{k}")
                  for k in range(2)]
            for k in range(2):
                sl = slice(128 * k, 128 * (k + 1))
                nc.sync.dma_start(out=wkv[k], in_=wkv_d[sl, :])
                nc.sync.dma_start(out=wq[k], in_=wq_d[sl, :])
                nc.sync.dma_start(out=wo[k], in_=wo_d[sl, :])
                nc.sync.dma_start(out=wp[k], in_=wp_d[sl, :])
            rblk = load("rblk", rblk_d, [128, 128])
            hmask = load("hmask", hmask_d, [128, 128])
            id16 = load("id16", id16_d, [128, 128])
            wl6 = load("wl6", wl6_d, [128, 6], dt.float32)

            # x tiles, split loads so chunk 0 can start early
            xct = [const.tile([128, TH], dt.bfloat16, tag=f"xct{k}",
                              name=f"xct{k}") for k in range(2)]
            XSPL = [0, 1026, 2050, 3074, TH]
            for k in range(2):
                for si in range(4):
                    cs = slice(XSPL[si], XSPL[si + 1])
                    nc.sync.dma_start(out=xct[k][:, cs],
                                      in_=xct_d[128 * k:128 * (k + 1), cs])
            srep = const.tile([128, T], dt.bfloat16, tag="srep", name="srep")
            crep = const.tile([128, T], dt.bfloat16, tag="crep", name="crep")
            for si in range(2):
                cs = slice(si * (T // 2), (si + 1) * (T // 2))
                nc.sync.dma_start(out=srep[:, cs], in_=srep_d[:, cs])
                nc.sync.dma_start(out=crep[:, cs], in_=crep_d[:, cs])

            # persistent activations (paired layout: col c*1024 + j*512 + t)
            q1p = const.tile([128, 2 * T], dt.bfloat16, tag="q1p", name="q1p")
            qsp = const.tile([128, 2 * T], dt.bfloat16, tag="qsp", name="qsp")
            o1p = const.tile([128, 2 * T], dt.bfloat16, tag="o1p", name="o1p")
            vT = [const.tile([128, TH], dt.bfloat16, tag=f"vT{j}", name=f"vT{j}")
                  for j in range(2)]
            kpart = const.tile([128, 16], dt.float32, tag="kpart", name="kpart")
            vpart = const.tile([128, 16], dt.float32, tag="vpart", name="vpart")
            stats = const.tile([128, 68], dt.float32, tag="stats", name="stats")
            stats2 = const.tile([128, 68], dt.float32, tag="stats2", name="stats2")

            # psum: "pj" 4 banks, "rot" 2 banks, "tp" 2x bf16 (1 bank), "acc" 1
            def pj(nm):
                return ppool.tile([128, 512], dt.float32, tag="pj", bufs=3,
                                  name=nm)

            def rot(nm):
                return ppool.tile([128, 512], dt.float32, tag="rot", bufs=2,
                                  name=nm)

            def tpt(nm):
                return ppool.tile([128, 512], dt.bfloat16, tag="tp", bufs=2,
                                  name=nm)

            acc = ppool.tile([128, 512], dt.float32, tag="acc", bufs=1,
                             name="acc")
            gram = acc[:, 0:256]

            def wt(nm, dtype=dt.bfloat16, cols=512, bufs=2):
                return work.tile([128, cols], dtype, tag=nm, bufs=bufs, name=nm)

            # =========================== phase A (k, v) ====================
            stA = {}

            def emit_projA(c):
                xs = [x_[:, 1 + c * CH: 1 + (c + 1) * CH] for x_ in xct]
                kps, vps = [], []
                for j in range(2):
                    p = pj(f"kps{j}_{c}")
                    nc.tensor.matmul(p, wkv[0][:, 128 * j:128 * (j + 1)], xs[0],
                                     start=True, stop=False)
                    nc.tensor.matmul(p, wkv[1][:, 128 * j:128 * (j + 1)], xs[1],
                                     start=False, stop=True)
                    kps.append(p)
                for j in range(2):
                    p = pj(f"vps{j}_{c}")
                    vsl = slice(256 + 128 * j, 256 + 128 * (j + 1))
                    nc.tensor.matmul(p, wkv[0][:, vsl], xs[0],
                                     start=True, stop=False)
                    nc.tensor.matmul(p, wkv[1][:, vsl], xs[1],
                                     start=False, stop=True)
                    vps.append(p)
                stA[c] = (kps, vps)

            def emit_tailA(c):
                kps, vps = stA.pop(c)
                ssl = srep[:, c * CH:(c + 1) * CH]
                csl = crep[:, c * CH:(c + 1) * CH]
                k1s = []
                for j in range(2):
                    # elu(k)+1 = min(exp(k),1) + relu(k); ksum rides the stt
                    e = wt("e")
                    nc.scalar.activation(e, kps[j], AF.Exp)
                    r = wt("r")
                    nc.scalar.activation(r, kps[j], AF.Relu)
                    k1 = wt("k1")
                    nc.vector.scalar_tensor_tensor(
                        out=k1, in0=e, scalar=1.0, in1=r,
                        op0=OP.min, op1=OP.add,
                        accum_out=kpart[:, 8 * j + c: 8 * j + c + 1])
                    k1s.append(k1)
                ks = []
                for j in range(2):
                    k1 = k1s[j]
                    # v evac (vsum rides accum)
                    nc.scalar.activation(
                        vT[j][:, 1 + c * CH: 1 + (c + 1) * CH], vps[j],
                        AF.Copy, accum_out=vpart[:, 8 * j + c: 8 * j + c + 1])
                    # rope(k)
                    rk = rot(f"rk{j}_{c}")
                    nc.tensor.matmul(rk, rblk, k1, start=True, stop=True)
                    m2 = wt("m2")
                    nc.vector.tensor_mul(m2, rk, ssl)
                    m1 = wt("m1")
                    nc.gpsimd.tensor_mul(m1, k1, csl)
                    ksj = wt(f"ks{j}")
                    nc.vector.tensor_add(ksj, m1, m2)
                    ks.append(ksj)
                # transpose + gram accumulation
                for s in range(4):
                    ktp = tpt(f"ktp{c}_{s}")
                    nc.tensor.transpose(ktp[:, 0:128],
                                        ks[0][:, s * 128:(s + 1) * 128], id16)
                    nc.tensor.transpose(ktp[:, 128:256],
                                        ks[1][:, s * 128:(s + 1) * 128], id16)
                    vcol = 1 + c * CH + s * 128
                    nc.tensor.transpose(ktp[:, 256:384],
                                        vT[0][:, vcol:vcol + 128], id16)
                    nc.tensor.transpose(ktp[:, 384:512],
                                        vT[1][:, vcol:vcol + 128], id16)
                    kvtok = wt("kvtok")
                    if s % 2 == 0:
                        nc.scalar.activation(kvtok, ktp, AF.Copy)
                    else:
                        nc.vector.tensor_copy(kvtok, ktp)
                    # PSUM start=True resets at coarser-than-region
                    # granularity: only the very first matmul into this bank
                    # may carry start=True; the second column group
                    # accumulates onto the zeros that reset left behind.
                    first = (c == 0 and s == 0)
                    last = (c == NCH - 1 and s == 3)
                    nc.tensor.matmul(gram[:, 0:128], kvtok[:, 0:128],
                                     kvtok[:, 256:384], start=first, stop=False)
                    nc.tensor.matmul(gram[:, 128:256], kvtok[:, 128:256],
                                     kvtok[:, 384:512], start=False, stop=last)

            for c in range(NCH):
                emit_projA(c)
                if c > 0:
                    emit_tailA(c - 1)
            emit_tailA(NCH - 1)

            # ================= stats pack + AllReduce ======================
            # gram diag head-blocks -> stats[:, 32j:32j+32]
            for j in range(2):
                for a in range(4):
                    psl = slice(32 * a, 32 * (a + 1))
                    nc.vector.tensor_copy(
                        stats[psl, 32 * j:32 * (j + 1)],
                        gram[psl, 128 * j + 32 * a:128 * j + 32 * (a + 1)])
            nc.vector.tensor_reduce(stats[:, 64:65], kpart[:, 0:8],
                                    axis=mybir.AxisListType.X, op=OP.add)
            nc.vector.tensor_reduce(stats[:, 65:66], kpart[:, 8:16],
                                    axis=mybir.AxisListType.X, op=OP.add)
            nc.vector.tensor_reduce(stats[:, 66:67], vpart[:, 0:8],
                                    axis=mybir.AxisListType.X, op=OP.add)
            nc.vector.tensor_reduce(stats[:, 67:68], vpart[:, 8:16],
                                    axis=mybir.AxisListType.X, op=OP.add)

            if nocc:
                nc.vector.tensor_scalar_mul(stats2, stats, 1.0)
            else:
                ccin = dpool.tile([128, 68], dt.float32, tag="ccin", name="ccin")
                ccout = dpool.tile([128, 68], dt.float32, tag="ccout",
                                   name="ccout")
                nc.gpsimd.dma_start(out=ccin[:, :], in_=stats)
                nc.gpsimd.collective_compute(
                    "AllReduce", OP.add, replica_groups=RG,
                    ins=[ccin[:, :]], outs=[ccout[:, :]])
                nc.gpsimd.dma_start(out=stats2, in_=ccout[:, :])

            # =========================== phase B (q, o) ====================
            # halo v columns (tokens t0-1 and t0+T) for the conv; single
            # accumulation group (one start, one stop).  Lives in the second
            # half-bank of acc: PSUM start=True resets a 1 KB half-bank, so
            # it must not share a half-bank with the live gram.
            hal = acc[:, 384:400]
            for j in range(2):
                vsl = slice(256 + 128 * j, 256 + 128 * (j + 1))
                cl = slice(j * 4, j * 4 + 1)
                cr = slice(j * 4 + 2, j * 4 + 3)
                nc.tensor.matmul(hal[:, cl], wkv[0][:, vsl], xct[0][:, 0:1],
                                 start=(j == 0), stop=False)
                nc.tensor.matmul(hal[:, cl], wkv[1][:, vsl], xct[1][:, 0:1],
                                 start=False, stop=False)
                nc.tensor.matmul(hal[:, cr], wkv[0][:, vsl],
                                 xct[0][:, TH - 1:TH], start=False, stop=False)
                nc.tensor.matmul(hal[:, cr], wkv[1][:, vsl],
                                 xct[1][:, TH - 1:TH], start=False,
                                 stop=(j == 1))
            for j in range(2):
                nc.scalar.activation(vT[j][:, 0:1],
                                     hal[:, j * 4:j * 4 + 1], AF.Copy)
                nc.scalar.activation(vT[j][:, TH - 1:TH],
                                     hal[:, j * 4 + 2:j * 4 + 3], AF.Copy)

            stB = {}

            def emit_projB(c):
                xs = [x_[:, 1 + c * CH: 1 + (c + 1) * CH] for x_ in xct]
                qps, ops_ = [], []
                for j in range(2):
                    p = pj(f"qps{j}_{c}")
                    nc.tensor.matmul(p, wq[0][:, 128 * j:128 * (j + 1)], xs[0],
                                     start=True, stop=False)
                    nc.tensor.matmul(p, wq[1][:, 128 * j:128 * (j + 1)], xs[1],
                                     start=False, stop=True)
                    qps.append(p)
                for j in range(2):
                    p = pj(f"ops{j}_{c}")
                    nc.tensor.matmul(p, wo[0][:, 128 * j:128 * (j + 1)], xs[0],
                                     start=True, stop=False)
                    nc.tensor.matmul(p, wo[1][:, 128 * j:128 * (j + 1)], xs[1],
                                     start=False, stop=True)
                    ops_.append(p)
                stB[c] = (qps, ops_)

            def emit_tailB(c):
                qps, ops_ = stB.pop(c)
                ssl = srep[:, c * CH:(c + 1) * CH]
                csl = crep[:, c * CH:(c + 1) * CH]
                for j in range(2):
                    gsl = slice(c * 1024 + j * 512, c * 1024 + (j + 1) * 512)
                    e = wt("e")
                    nc.scalar.activation(e, qps[j], AF.Exp)
                    r = wt("r")
                    nc.scalar.activation(r, qps[j], AF.Relu)
                    nc.vector.scalar_tensor_tensor(
                        out=q1p[:, gsl], in0=e, scalar=1.0, in1=r,
                        op0=OP.min, op1=OP.add)
                    nc.scalar.activation(o1p[:, gsl], ops_[j], AF.Copy)
                    rq = rot(f"rq{j}_{c}")
                    nc.tensor.matmul(rq, rblk, q1p[:, gsl],
                                     start=True, stop=True)
                    m2 = wt("m2")
                    nc.vector.tensor_mul(m2, rq, ssl)
                    m1 = wt("m1")
                    nc.gpsimd.tensor_mul(m1, q1p[:, gsl], csl)
                    nc.vector.tensor_add(qsp[:, gsl], m1, m2)

            for c in range(NCH):
                emit_projB(c)
                if c > 0:
                    emit_tailB(c - 1)
            emit_tailB(NCH - 1)

            # ================= phase C consts (post-AR) ====================
            zsc = const.tile([128, 2], dt.float32, tag="zsc", name="zsc")
            nc.scalar.mul(zsc, stats2[:, 64:66], SCALE / N)
            vmn = const.tile([128, 2], dt.float32, tag="vmn", name="vmn")
            nc.scalar.mul(vmn, stats2[:, 66:68], -1.0 / N)
            zblk, kvblk = [], []
            for j in range(2):
                zb = const.tile([128, 128], dt.bfloat16, tag=f"zb{j}",
                                name=f"zb{j}")
                nc.vector.tensor_tensor(
                    zb, zsc[:, j:j + 1].to_broadcast((128, 128)), hmask,
                    OP.mult)
                zblk.append(zb)
                kvb = const.tile([128, 128], dt.bfloat16, tag=f"kvb{j}",
                                 name=f"kvb{j}")
                nc.vector.memset(kvb, 0.0)
                for a in range(4):
                    psl = slice(32 * a, 32 * (a + 1))
                    nc.scalar.mul(kvb[psl, psl],
                                  stats2[psl, 32 * j:32 * (j + 1)], KSC)
                kvblk.append(kvb)

            # =========================== phase C ===========================
            stC = {}

            def emit_mmC(c):
                zps, aps = [], []
                for j in range(2):
                    gsl = slice(c * 1024 + j * 512, c * 1024 + (j + 1) * 512)
                    zp = pj(f"zps{j}_{c}")
                    nc.tensor.matmul(zp, zblk[j], q1p[:, gsl],
                                     start=True, stop=True)
                    zps.append(zp)
                    ap = pj(f"aps{j}_{c}")
                    nc.tensor.matmul(ap, kvblk[j], qsp[:, gsl],
                                     start=True, stop=True)
                    aps.append(ap)
                stC[c] = (zps, aps)

            def emit_tailC(c):
                zps, aps = stC.pop(c)
                ys = []
                for j in range(2):
                    gsl = slice(c * 1024 + j * 512, c * 1024 + (j + 1) * 512)
                    rz = wt("rz", dt.float32)
                    nc.vector.reciprocal_approx_fast(out=rz, in_=zps[j])
                    # corr = -z*vmean (frees the zps bank early)
                    c0 = wt("c0")
                    nc.vector.tensor_scalar(
                        out=c0, in0=zps[j], scalar1=vmn[:, j:j + 1],
                        scalar2=None, op0=OP.mult)
                    a1 = wt("a1")
                    nc.vector.scalar_tensor_tensor(
                        out=a1, in0=rz, scalar=1.0, in1=aps[j],
                        op0=OP.add, op1=OP.mult)
                    # lepe: 3 taps, per-partition weights, shifted v slices
                    s1 = wt("s1")
                    nc.vector.tensor_scalar(
                        out=s1, in0=vT[j][:, c * CH:c * CH + 512],
                        scalar1=wl6[:, 3 * j:3 * j + 1], scalar2=None,
                        op0=OP.mult)
                    s2 = wt("s2")
                    nc.vector.scalar_tensor_tensor(
                        out=s2, in0=vT[j][:, c * CH + 1:c * CH + 513],
                        scalar=wl6[:, 3 * j + 1:3 * j + 2], in1=s1,
                        op0=OP.mult, op1=OP.add)
                    s3 = wt("s3")
                    nc.vector.scalar_tensor_tensor(
                        out=s3, in0=vT[j][:, c * CH + 2:c * CH + 514],
                        scalar=wl6[:, 3 * j + 2:3 * j + 3], in1=s2,
                        op0=OP.mult, op1=OP.add)
                    r1 = wt("r1")
                    nc.vector.tensor_add(r1, a1, c0)
                    r2 = wt("r2")
                    nc.vector.tensor_add(r2, r1, s3)
                    yj = wt(f"y{j}")
                    nc.gpsimd.tensor_mul(yj, r2, o1p[:, gsl])
                    ys.append(yj)
                outp = [rot(f"outp{h}_{c}") for h in range(2)]
                for s in range(4):
                    osl = slice((s % 2) * 256, (s % 2) * 256 + 256)
                    nc.tensor.matmul(outp[s // 2][:, osl],
                                     ys[0][:, s * 128:(s + 1) * 128], wp[0],
                                     start=True, stop=False)
                    nc.tensor.matmul(outp[s // 2][:, osl],
                                     ys[1][:, s * 128:(s + 1) * 128], wp[1],
                                     start=False, stop=True)
                outsb = wt("outsb", dt.float32, cols=1024)
                nc.scalar.activation(outsb[:, 0:512], outp[0], AF.Copy)
                nc.scalar.activation(outsb[:, 512:1024], outp[1], AF.Copy)
                dsl = out_d[c * CH: (c + 1) * CH, :]
                nc.sync.dma_start(out=dsl.rearrange("(s t) o -> t s o", s=4),
                                  in_=outsb)

            for c in range(NCH):
                emit_mmC(c)
                if c > 0:
                    emit_tailC(c - 1)
            emit_tailC(NCH - 1)

            if dbg:
                nc.sync.dma_start(out=dbg32_d[:, :], in_=stats2)
                nc.sync.dma_start(out=dbg16_d[:, 0:1024], in_=q1p[:, 0:1024])
                nc.sync.dma_start(out=dbg16_d[:, 1024:2048], in_=qsp[:, 0:1024])
                nc.sync.dma_start(out=dbg16_d[:, 2048:3072], in_=o1p[:, 0:1024])
                nc.sync.dma_start(out=dbg16_d[:, 3072:4096],
                                  in_=vT[0][:, 0:1024])

    nc.compile()
    return nc


_NC_CACHE = {}


def _get_nc():
    key = (bool(os.environ.get("KERNEL_NOCC")),
           bool(os.environ.get("KERNEL_DBG")))
    if key not in _NC_CACHE:
        _NC_CACHE[key] = _build_nc()
    return _NC_CACHE[key]


def kernel(x, sin, cos, W_qkvo, b_qkvo, W_lepe, b_lepe, W_proj, b_proj):
    from concourse.bass_utils import run_bass_kernel_spmd

    per_core = _host_prep(x, sin, cos, W_qkvo, b_qkvo, W_lepe, b_lepe,
                          W_proj, b_proj)
    nc = _get_nc()
    # keep only the inputs that survived DCE in the compiled program
    import concourse.mybir as mybir
    expected = set()
    for alloc in nc.m.functions[0].allocations:
        if isinstance(alloc, mybir.MemoryLocationSet) and alloc.kind == "ExternalInput":
            expected.add(alloc.memorylocations[0].name)
    per_core = [{k: v for k, v in m.items() if k in expected} for m in per_core]
    res = run_bass_kernel_spmd(nc, per_core, core_ids=list(range(NCORES)),
                               trace=bool(os.environ.get("KERNEL_TRACE")))
    if os.environ.get("KERNEL_TRACE"):
        kernel.last_exec_time_ns = res.exec_time_ns
        kernel.last_results = res
    full = np.zeros((B, N, INTERNAL), np.float32)
    for c in range(NCORES):
        b = c // 2
        t0 = (c % 2) * T
        full[b, t0:t0 + T] = res.results[c]["out"]
    return full


# ---------------------------------------------------------- numpy selftest
# numpy emulation of the exact device pipeline (fp32), validates the
# decomposition (run with KERNEL_SELFTEST=1).

def _numpy_pipeline(per_core_inputs):
    cores = []
    for c in range(NCORES):
        d = per_core_inputs[c]
        xct = d["xct"].astype(np.float32)          # [256, TH]
        srep = d["srep"].astype(np.float32)
        crep = d["crep"].astype(np.float32)
        wq = d["wq"].astype(np.float32)
        wkv = d["wkv"].astype(np.float32)
        wo = d["wo"].astype(np.float32)
        R = d["rblk"].astype(np.float32)

        x_in = xct[:, 1:T + 1]                     # [256, T]
        qT = wq.T @ x_in                           # [256, T]
        kT = wkv[:, 0:256].T @ x_in
        vT_m = wkv[:, 256:512].T @ x_in
        oT = wo.T @ x_in
        vhl = wkv[:, 256:512].T @ xct[:, 0:1]
        vhr = wkv[:, 256:512].T @ xct[:, TH - 1:TH]
        vT = np.concatenate([vhl, vT_m, vhr], axis=1)      # [256, TH]

        def elu1(t):
            return np.minimum(np.exp(t), 1.0) + np.maximum(t, 0.0)

        q1 = elu1(qT)
        k1 = elu1(kT)

        ks = np.zeros_like(k1)
        qs = np.zeros_like(q1)
        for j in range(2):
            sl = slice(128 * j, 128 * (j + 1))
            ks[sl] = k1[sl] * crep + (R.T @ k1[sl]) * srep
            qs[sl] = q1[sl] * crep + (R.T @ q1[sl]) * srep

        gram = np.zeros((128, 256), np.float32)
        for j in range(2):
            gram[:, 128 * j:128 * (j + 1)] = (
                ks[128 * j:128 * (j + 1)] @ vT[128 * j:128 * (j + 1), 1:T + 1].T)
        ksum = k1.sum(axis=1)                      # [256]
        vsum = vT[:, 1:T + 1].sum(axis=1)
        cores.append(dict(d=d, q1=q1, qs=qs, oT=oT, vT=vT, gram=gram,
                          ksum=ksum, vsum=vsum))

    for pair in range(4):
        a, b2 = cores[2 * pair], cores[2 * pair + 1]
        gram = a["gram"] + b2["gram"]
        ksum = a["ksum"] + b2["ksum"]
        vsum = a["vsum"] + b2["vsum"]
        for cc in (a, b2):
            cc["gram_r"], cc["ksum_r"], cc["vsum_r"] = gram, ksum, vsum

    outs = []
    for c in range(NCORES):
        st = cores[c]
        d = st["d"]
        q1, qs, oT, vT = st["q1"], st["qs"], st["oT"], st["vT"]
        hmask = d["hmask"].astype(np.float32)
        wl6 = d["wl6"].astype(np.float32)
        wp = d["wp"].astype(np.float32)
        gram, ksum, vsum = st["gram_r"], st["ksum_r"], st["vsum_r"]

        res = np.zeros((256, T), np.float32)
        for j in range(2):
            sl = slice(128 * j, 128 * (j + 1))
            zsc = (SCALE / N) * ksum[sl]                     # [128]
            zblk = zsc[:, None] * hmask                      # [128,128]
            zps = zblk.T @ q1[sl]                            # [128, T]
            kvblk = np.zeros((128, 128), np.float32)
            for aa in range(4):
                s2 = slice(32 * aa, 32 * (aa + 1))
                kvblk[s2, s2] = KSC * gram[s2, 128 * j + 32 * aa:
                                           128 * j + 32 * (aa + 1)]
            aps = kvblk.T @ qs[sl]
            a1 = (1.0 / zps + 1.0) * aps
            lepe = (wl6[:, 3 * j:3 * j + 1] * vT[sl, 0:T]
                    + wl6[:, 3 * j + 1:3 * j + 2] * vT[sl, 1:T + 1]
                    + wl6[:, 3 * j + 2:3 * j + 3] * vT[sl, 2:T + 2])
            c1 = zps * (-vsum[sl] / N)[:, None] + lepe
            res[sl] = a1 + c1
        y = res * oT
        outs.append((y.T @ wp).astype(np.float32))

    full = np.zeros((B, N, 256), np.float32)
    for c in range(NCORES):
        b = c // 2
        t0 = (c % 2) * T
        full[b, t0:t0 + T] = outs[c]
    return full


if __name__ == "__main__" and os.environ.get("KERNEL_SELFTEST"):
    sys.path.insert(0, os.path.dirname(os.path.abspath(__file__)))
    import reference
    inputs = {k: np.asarray(v) for k, v in reference.setup_inputs().items()}
    expected = np.asarray(reference.reference(**inputs))
    per_core = _host_prep(**inputs)
    got = _numpy_pipeline(per_core)
    rel = np.linalg.norm(got - expected) / np.linalg.norm(expected)
    print("selftest rel err:", rel, "max abs:", np.abs(got - expected).max())

if __name__ == "__main__" and os.environ.get("KERNEL_BUILD"):
    nc = _build_nc()
    print("build OK")


# revision 24
# speedup vs baseline: 1.1419x; 1.1419x over previous
"""Trainium2 Bass kernel for nn_MultiHeadMALAAttention.

Sharding: 8 cores; core c handles batch b = c//2, token half h = c%2
(tokens [h*4096, (h+1)*4096) of N=8192).  Stats (ksum, vsum, kv gram)
need full-N reductions -> pairwise AllReduce between the two cores of a
batch, replica groups [[0,1],[2,3],[4,5],[6,7]].

Structure (per core), ordered so the AllReduce hides under compute:
  dummy 32B AllReduce first (absorbs the one-time CC barrier under the
  initial weight/x DMAs)
  phase A: k,v projections -> elu(k)+1 -> rope(k) -> PE transposes ->
           kv gram accumulation in PSUM; ksum/vsum ride accum_out.
  stats pack ([128,68]) -> AllReduce
  phase B: q,o projections -> elu(q)+1 -> rope(q) -> qs  (runs during AR)
  phase C: z = zblk^T q1; attn = kvblk^T qs; then everything else on DVE:
           res = (1+1/z)*attn - z*vmean + lepe(v), y = res*o, out proj.

Key fusions vs a naive pipeline: elu+1 = min(exp(x),1)+relu(x) (2 ACT +
1 stt); the (1+1/z) factor commutes past the per-head kv matmul so
rope(q) is stats-independent; -z*vmean rides the z PSUM as one stt; the
depthwise conv (LEPE) is 3 stt ops with per-partition channel weights.
"""

import os
import sys

sys.path.insert(0, "/opt/trn_rl_repo")

import numpy as np
import ml_dtypes

B, N, DIM, H, HD = 4, 8192, 256, 8, 32
INTERNAL = H * HD  # 256
SCALE = HD ** -0.5
NCORES = 8
T = N // 2          # tokens per core
TH = T + 2          # with 1-token halo each side
CH = 512            # chunk tokens
NCH = T // CH       # chunks per core
KSC = SCALE / N     # kv_state scale (s^2)

BF16 = ml_dtypes.bfloat16


# ---------------------------------------------------------------- host prep

def _host_prep(x, sin, cos, W_qkvo, b_qkvo, W_lepe, b_lepe, W_proj, b_proj):
    """Build per-core input dicts (all device tensors)."""
    assert not (np.any(b_qkvo) or np.any(b_lepe) or np.any(b_proj)), \
        "bias path not implemented (setup_inputs uses zero biases)"

    WT = W_qkvo.T.astype(np.float32)          # [DIM, 1024] = lhsT layout
    wq = WT[:, 0:256].astype(BF16)
    wkv = WT[:, 256:768].astype(BF16)          # k cols 0:256, v cols 256:512
    wo = WT[:, 768:1024].astype(BF16)
    wp = W_proj.T.astype(np.float32).astype(BF16)   # [DIM, 256] rhs layout
    wl = W_lepe[:, 0, :].astype(np.float32)    # [256, 3]
    # per-partition lepe weights: tile j cols 3j..3j+3
    wl6 = np.zeros((128, 6), np.float32)
    wl6[:, 0:3] = wl[0:128]
    wl6[:, 3:6] = wl[128:256]

    # diag conv weights: block (tap j, tile m) = diag(wl[128m:128(m+1), j])
    dcw = np.zeros((128, 6, 128), np.float32)
    for j in range(3):
        for m in range(2):
            np.fill_diagonal(dcw[:, j * 2 + m, :], wl[128 * m:128 * (m + 1), j])
    dcw = dcw.reshape(128, 768).astype(BF16)

    # rotate-every-two matrix as lhsT: rot = R.T @ x ; R[k, m] = coeff of
    # chan k in rot-chan m:  rot[2i] = -x[2i+1], rot[2i+1] = x[2i]
    R = np.zeros((128, 128), np.float32)
    for i in range(64):
        R[2 * i + 1, 2 * i] = -1.0
        R[2 * i, 2 * i + 1] = 1.0
    R = R.astype(BF16)

    hmask = np.zeros((128, 128), np.float32)
    for hh in range(4):
        hmask[32 * hh:32 * (hh + 1), 32 * hh:32 * (hh + 1)] = 1.0
    hmask = hmask.astype(BF16)

    ident16 = np.eye(128, dtype=np.float32).astype(BF16)
    ident32 = np.eye(128, dtype=np.float32)

    xf = np.asarray(x, np.float32)
    sinf = np.asarray(sin, np.float32)
    cosf = np.asarray(cos, np.float32)

    per_core = []
    for c in range(NCORES):
        b = c // 2
        t0 = (c % 2) * T
        # x channel-major with halo [256, TH]
        xpad = np.zeros((TH, DIM), np.float32)
        lo, hi = t0 - 1, t0 + T + 1
        slo, shi = max(lo, 0), min(hi, N)
        xpad[slo - lo: slo - lo + (shi - slo)] = xf[b, slo:shi]
        xct = np.ascontiguousarray(xpad.T).astype(BF16)          # [256, TH]

        srep = np.tile(sinf[t0:t0 + T].T, (4, 1)).astype(BF16)   # [128, T]
        crep = np.tile(cosf[t0:t0 + T].T, (4, 1)).astype(BF16)   # [128, T]

        per_core.append({
            "xct": xct, "srep": np.ascontiguousarray(srep),
            "crep": np.ascontiguousarray(crep),
            "wq": np.ascontiguousarray(wq), "wkv": np.ascontiguousarray(wkv),
            "wo": np.ascontiguousarray(wo), "wp": np.ascontiguousarray(wp),
            "rblk": R, "hmask": hmask, "ident16": ident16, "wl6": wl6,
            "ident32": ident32, "dcw": dcw,
        })
    return per_core


# ------------------------------------------------------------ device kernel

def _build_nc():
    from concourse import bacc
    import concourse.mybir as mybir
    import concourse.tile as tile

    dt = mybir.dt
    AF = mybir.ActivationFunctionType
    OP = mybir.AluOpType

    nocc = bool(os.environ.get("KERNEL_NOCC"))

    nc = bacc.Bacc(None, target_bir_lowering=False)

    # ---- I/O
    xct_d = nc.dram_tensor("xct", [256, TH], dt.bfloat16, kind="ExternalInput")
    srep_d = nc.dram_tensor("srep", [128, T], dt.bfloat16, kind="ExternalInput")
    crep_d = nc.dram_tensor("crep", [128, T], dt.bfloat16, kind="ExternalInput")
    wq_d = nc.dram_tensor("wq", [256, 256], dt.bfloat16, kind="ExternalInput")
    wkv_d = nc.dram_tensor("wkv", [256, 512], dt.bfloat16, kind="ExternalInput")
    wo_d = nc.dram_tensor("wo", [256, 256], dt.bfloat16, kind="ExternalInput")
    wp_d = nc.dram_tensor("wp", [256, 256], dt.bfloat16, kind="ExternalInput")
    rblk_d = nc.dram_tensor("rblk", [128, 128], dt.bfloat16, kind="ExternalInput")
    hmask_d = nc.dram_tensor("hmask", [128, 128], dt.bfloat16, kind="ExternalInput")
    id16_d = nc.dram_tensor("ident16", [128, 128], dt.bfloat16, kind="ExternalInput")
    id32_d = nc.dram_tensor("ident32", [128, 128], dt.float32, kind="ExternalInput")
    dcw_d = nc.dram_tensor("dcw", [128, 768], dt.bfloat16, kind="ExternalInput")
    out_d = nc.dram_tensor("out", [T, 256], dt.float32, kind="ExternalOutput")
    dbg = bool(os.environ.get("KERNEL_DBG"))
    if dbg:
        dbg16_d = nc.dram_tensor("dbg16", [128, 4096], dt.bfloat16,
                                 kind="ExternalOutput")
        dbg32_d = nc.dram_tensor("dbg32", [128, 68], dt.float32,
                                 kind="ExternalOutput")

    RG = [[0, 1], [2, 3], [4, 5], [6, 7]]

    with tile.TileContext(nc) as tc:
        with (
            tc.tile_pool(name="const", bufs=1) as const,
            tc.tile_pool(name="work", bufs=2) as work,
            tc.tile_pool(name="psum", bufs=1, space="PSUM") as ppool,
            tc.tile_pool(name="dram", bufs=1, space="DRAM") as dpool,
        ):
            # ---- dummy collective: absorb the one-time CC barrier early
            if not nocc:
                dum = const.tile([1, 8], dt.float32, tag="dum", name="dum")
                nc.vector.memset(dum, 0.0)
                ccd_i = dpool.tile([1, 8], dt.float32, tag="ccdi", name="ccdi")
                ccd_o = dpool.tile([1, 8], dt.float32, tag="ccdo", name="ccdo")
                nc.gpsimd.dma_start(out=ccd_i[:, :], in_=dum)
                nc.gpsimd.collective_compute(
                    "AllReduce", OP.add, replica_groups=RG,
                    ins=[ccd_i[:, :]], outs=[ccd_o[:, :]])

            # ---- consts
            def load(tname, dten, shape, dtype=dt.bfloat16):
                t_ = const.tile(shape, dtype, tag=tname, name=tname)
                nc.sync.dma_start(out=t_, in_=dten[:, :])
                return t_

            wq = [const.tile([128, 256], dt.bfloat16, tag=f"wq{k}", name=f"wq{k}")
                  for k in range(2)]
            wkv = [const.tile([128, 512], dt.bfloat16, tag=f"wkv{k}", name=f"wkv{k}")
                   for k in range(2)]
            wo = [const.tile([128, 256], dt.bfloat16, tag=f"wo{k}", name=f"wo{k}")
                  for k in range(2)]
            wp = [const.tile([128, 256], dt.bfloat16, tag=f"wp{k}", name=f"wp{k}")
                  for k in range(2)]
            for k in range(2):
                sl = slice(128 * k, 128 * (k + 1))
                nc.sync.dma_start(out=wkv[k], in_=wkv_d[sl, :])
                nc.sync.dma_start(out=wq[k], in_=wq_d[sl, :])
                nc.sync.dma_start(out=wo[k], in_=wo_d[sl, :])
                nc.sync.dma_start(out=wp[k], in_=wp_d[sl, :])
            rblk = load("rblk", rblk_d, [128, 128])
            hmask = load("hmask", hmask_d, [128, 128])
            id16 = load("id16", id16_d, [128, 128])
            id32 = load("id32", id32_d, [128, 128], dt.float32)
            dcw = load("dcw", dcw_d, [128, 768])

            # x tiles, split loads so chunk 0 can start early
            xct = [const.tile([128, TH], dt.bfloat16, tag=f"xct{k}",
                              name=f"xct{k}") for k in range(2)]
            XSPL = [0, 1026, 2050, 3074, TH]
            for k in range(2):
                for si in range(4):
                    cs = slice(XSPL[si], XSPL[si + 1])
                    nc.sync.dma_start(out=xct[k][:, cs],
                                      in_=xct_d[128 * k:128 * (k + 1), cs])
            srep = const.tile([128, T], dt.bfloat16, tag="srep", name="srep")
            crep = const.tile([128, T], dt.bfloat16, tag="crep", name="crep")
            for si in range(2):
                cs = slice(si * (T // 2), (si + 1) * (T // 2))
                nc.sync.dma_start(out=srep[:, cs], in_=srep_d[:, cs])
                nc.sync.dma_start(out=crep[:, cs], in_=crep_d[:, cs])

            # persistent activations (paired layout: col c*1024 + j*512 + t)
            q1p = const.tile([128, 2 * T], dt.bfloat16, tag="q1p", name="q1p")
            qsp = const.tile([128, 2 * T], dt.bfloat16, tag="qsp", name="qsp")
            o1p = const.tile([128, 2 * T], dt.bfloat16, tag="o1p", name="o1p")
            vT = [const.tile([128, TH], dt.bfloat16, tag=f"vT{j}", name=f"vT{j}")
                  for j in range(2)]
            kpart = const.tile([128, 16], dt.float32, tag="kpart", name="kpart")
            vpart = const.tile([128, 16], dt.float32, tag="vpart", name="vpart")
            stats = const.tile([128, 68], dt.float32, tag="stats", name="stats")
            stats2 = const.tile([128, 68], dt.float32, tag="stats2", name="stats2")

            # psum: "pj" 4 banks, "rot" 2 banks, "tp" 2x bf16 (1 bank), "acc" 1
            def pj(nm):
                return ppool.tile([128, 512], dt.float32, tag="pj", bufs=3,
                                  name=nm)

            def rot(nm):
                return ppool.tile([128, 512], dt.float32, tag="rot", bufs=2,
                                  name=nm)

            def tpt(nm):
                return ppool.tile([128, 512], dt.bfloat16, tag="tp", bufs=2,
                                  name=nm)

            def tpt2(nm):
                return ppool.tile([128, 512], dt.float32, tag="tp", bufs=2,
                                  name=nm)

            acc = ppool.tile([128, 512], dt.float32, tag="acc", bufs=1,
                             name="acc")
            gram = acc[:, 0:256]

            def wt(nm, dtype=dt.bfloat16, cols=512, bufs=2):
                return work.tile([128, cols], dtype, tag=nm, bufs=bufs, name=nm)

            # =========================== phase A (k, v) ====================
            stA = {}

            def emit_projA(c):
                xs = [x_[:, 1 + c * CH: 1 + (c + 1) * CH] for x_ in xct]
                kps, vps = [], []
                for j in range(2):
                    p = pj(f"kps{j}_{c}")
                    nc.tensor.matmul(p, wkv[0][:, 128 * j:128 * (j + 1)], xs[0],
                                     start=True, stop=False)
                    nc.tensor.matmul(p, wkv[1][:, 128 * j:128 * (j + 1)], xs[1],
                                     start=False, stop=True)
                    kps.append(p)
                for j in range(2):
                    p = pj(f"vps{j}_{c}")
                    vsl = slice(256 + 128 * j, 256 + 128 * (j + 1))
                    nc.tensor.matmul(p, wkv[0][:, vsl], xs[0],
                                     start=True, stop=False)
                    nc.tensor.matmul(p, wkv[1][:, vsl], xs[1],
                                     start=False, stop=True)
                    vps.append(p)
                stA[c] = (kps, vps)

            def emit_tailA(c):
                kps, vps = stA.pop(c)
                ssl = srep[:, c * CH:(c + 1) * CH]
                csl = crep[:, c * CH:(c + 1) * CH]
                k1s = []
                for j in range(2):
                    # elu(k)+1 = min(exp(k),1) + relu(k); ksum rides the stt
                    e = wt("e")
                    nc.scalar.activation(e, kps[j], AF.Exp)
                    r = wt("r")
                    nc.scalar.activation(r, kps[j], AF.Relu)
                    k1 = wt("k1")
                    nc.vector.scalar_tensor_tensor(
                        out=k1, in0=e, scalar=1.0, in1=r,
                        op0=OP.min, op1=OP.add,
                        accum_out=kpart[:, 8 * j + c: 8 * j + c + 1])
                    k1s.append(k1)
                ks = []
                for j in range(2):
                    k1 = k1s[j]
                    # v evac (vsum rides accum)
                    nc.scalar.activation(
                        vT[j][:, 1 + c * CH: 1 + (c + 1) * CH], vps[j],
                        AF.Copy, accum_out=vpart[:, 8 * j + c: 8 * j + c + 1])
                    # rope(k)
                    rk = rot(f"rk{j}_{c}")
                    nc.tensor.matmul(rk, rblk, k1, start=True, stop=True)
                    m2 = wt("m2")
                    nc.vector.tensor_mul(m2, rk, ssl)
                    m1 = wt("m1")
                    nc.gpsimd.tensor_mul(m1, k1, csl)
                    ksj = wt(f"ks{j}")
                    nc.vector.tensor_add(ksj, m1, m2)
                    ks.append(ksj)
                # transpose + gram accumulation
                for s in range(4):
                    ktp = tpt(f"ktp{c}_{s}")
                    nc.tensor.transpose(ktp[:, 0:128],
                                        ks[0][:, s * 128:(s + 1) * 128], id16)
                    nc.tensor.transpose(ktp[:, 128:256],
                                        ks[1][:, s * 128:(s + 1) * 128], id16)
                    vcol = 1 + c * CH + s * 128
                    nc.tensor.transpose(ktp[:, 256:384],
                                        vT[0][:, vcol:vcol + 128], id16)
                    nc.tensor.transpose(ktp[:, 384:512],
                                        vT[1][:, vcol:vcol + 128], id16)
                    kvtok = wt("kvtok")
                    if s % 2 == 0:
                        nc.vector.tensor_copy(kvtok, ktp)
                    else:
                        nc.scalar.activation(kvtok, ktp, AF.Copy)
                    # PSUM start=True resets at coarser-than-region
                    # granularity: only the very first matmul into this bank
                    # may carry start=True; the second column group
                    # accumulates onto the zeros that reset left behind.
                    first = (c == 0 and s == 0)
                    last = (c == NCH - 1 and s == 3)
                    nc.tensor.matmul(gram[:, 0:128], kvtok[:, 0:128],
                                     kvtok[:, 256:384], start=first, stop=False)
                    nc.tensor.matmul(gram[:, 128:256], kvtok[:, 128:256],
                                     kvtok[:, 384:512], start=False, stop=last)

            for c in range(NCH):
                emit_projA(c)
                if c > 0:
                    emit_tailA(c - 1)
            emit_tailA(NCH - 1)

            # ================= stats pack + AllReduce ======================
            # gram diag head-blocks -> stats[:, 32j:32j+32]
            for j in range(2):
                for a in range(4):
                    psl = slice(32 * a, 32 * (a + 1))
                    nc.vector.tensor_copy(
                        stats[psl, 32 * j:32 * (j + 1)],
                        gram[psl, 128 * j + 32 * a:128 * j + 32 * (a + 1)])
            nc.vector.tensor_reduce(stats[:, 64:65], kpart[:, 0:8],
                                    axis=mybir.AxisListType.X, op=OP.add)
            nc.vector.tensor_reduce(stats[:, 65:66], kpart[:, 8:16],
                                    axis=mybir.AxisListType.X, op=OP.add)
            nc.vector.tensor_reduce(stats[:, 66:67], vpart[:, 0:8],
                                    axis=mybir.AxisListType.X, op=OP.add)
            nc.vector.tensor_reduce(stats[:, 67:68], vpart[:, 8:16],
                                    axis=mybir.AxisListType.X, op=OP.add)

            if nocc:
                nc.vector.tensor_scalar_mul(stats2, stats, 1.0)
            else:
                ccin = dpool.tile([128, 68], dt.float32, tag="ccin", name="ccin")
                ccout = dpool.tile([128, 68], dt.float32, tag="ccout",
                                   name="ccout")
                nc.gpsimd.dma_start(out=ccin[:, :], in_=stats)
                nc.gpsimd.collective_compute(
                    "AllReduce", OP.add, replica_groups=RG,
                    ins=[ccin[:, :]], outs=[ccout[:, :]])
                nc.gpsimd.dma_start(out=stats2, in_=ccout[:, :])

            # =========================== phase B (q, o) ====================
            # halo v columns (tokens t0-1 and t0+T) for the conv; single
            # accumulation group (one start, one stop).  Lives in the second
            # half-bank of acc: PSUM start=True resets a 1 KB half-bank, so
            # it must not share a half-bank with the live gram.
            hal = acc[:, 384:400]
            for j in range(2):
                vsl = slice(256 + 128 * j, 256 + 128 * (j + 1))
                cl = slice(j * 4, j * 4 + 1)
                cr = slice(j * 4 + 2, j * 4 + 3)
                nc.tensor.matmul(hal[:, cl], wkv[0][:, vsl], xct[0][:, 0:1],
                                 start=(j == 0), stop=False)
                nc.tensor.matmul(hal[:, cl], wkv[1][:, vsl], xct[1][:, 0:1],
                                 start=False, stop=False)
                nc.tensor.matmul(hal[:, cr], wkv[0][:, vsl],
                                 xct[0][:, TH - 1:TH], start=False, stop=False)
                nc.tensor.matmul(hal[:, cr], wkv[1][:, vsl],
                                 xct[1][:, TH - 1:TH], start=False,
                                 stop=(j == 1))
            for j in range(2):
                nc.scalar.activation(vT[j][:, 0:1],
                                     hal[:, j * 4:j * 4 + 1], AF.Copy)
                nc.scalar.activation(vT[j][:, TH - 1:TH],
                                     hal[:, j * 4 + 2:j * 4 + 3], AF.Copy)

            stB = {}

            def emit_projB(c):
                xs = [x_[:, 1 + c * CH: 1 + (c + 1) * CH] for x_ in xct]
                qps, ops_ = [], []
                for j in range(2):
                    p = pj(f"qps{j}_{c}")
                    nc.tensor.matmul(p, wq[0][:, 128 * j:128 * (j + 1)], xs[0],
                                     start=True, stop=False)
                    nc.tensor.matmul(p, wq[1][:, 128 * j:128 * (j + 1)], xs[1],
                                     start=False, stop=True)
                    qps.append(p)
                for j in range(2):
                    p = pj(f"ops{j}_{c}")
                    nc.tensor.matmul(p, wo[0][:, 128 * j:128 * (j + 1)], xs[0],
                                     start=True, stop=False)
                    nc.tensor.matmul(p, wo[1][:, 128 * j:128 * (j + 1)], xs[1],
                                     start=False, stop=True)
                    ops_.append(p)
                stB[c] = (qps, ops_)

            def emit_tailB(c):
                qps, ops_ = stB.pop(c)
                ssl = srep[:, c * CH:(c + 1) * CH]
                csl = crep[:, c * CH:(c + 1) * CH]
                for j in range(2):
                    gsl = slice(c * 1024 + j * 512, c * 1024 + (j + 1) * 512)
                    e = wt("e")
                    nc.scalar.activation(e, qps[j], AF.Exp)
                    r = wt("r")
                    nc.scalar.activation(r, qps[j], AF.Relu)
                    nc.vector.scalar_tensor_tensor(
                        out=q1p[:, gsl], in0=e, scalar=1.0, in1=r,
                        op0=OP.min, op1=OP.add)
                    nc.vector.tensor_copy(o1p[:, gsl], ops_[j])
                    rq = rot(f"rq{j}_{c}")
                    nc.tensor.matmul(rq, rblk, q1p[:, gsl],
                                     start=True, stop=True)
                    m2 = wt("m2")
                    nc.vector.tensor_mul(m2, rq, ssl)
                    m1 = wt("m1")
                    nc.gpsimd.tensor_mul(m1, q1p[:, gsl], csl)
                    nc.vector.tensor_add(qsp[:, gsl], m1, m2)

            for c in range(NCH):
                emit_projB(c)
                if c > 0:
                    emit_tailB(c - 1)
            emit_tailB(NCH - 1)

            # ================= phase C consts (post-AR) ====================
            zsc = const.tile([128, 2], dt.float32, tag="zsc", name="zsc")
            nc.scalar.mul(zsc, stats2[:, 64:66], SCALE / N)
            zblk, kvblk, mcorr = [], [], []
            for j in range(2):
                zb = const.tile([128, 128], dt.bfloat16, tag=f"zb{j}",
                                name=f"zb{j}")
                nc.vector.tensor_tensor(
                    zb, zsc[:, j:j + 1].to_broadcast((128, 128)), hmask,
                    OP.mult)
                zblk.append(zb)
                kvb = const.tile([128, 128], dt.bfloat16, tag=f"kvb{j}",
                                 name=f"kvb{j}")
                nc.vector.memset(kvb, 0.0)
                for a in range(4):
                    psl = slice(32 * a, 32 * (a + 1))
                    nc.scalar.mul(kvb[psl, psl],
                                  stats2[psl, 32 * j:32 * (j + 1)], KSC)
                kvblk.append(kvb)
                # mcorr[d,e] = -SCALE*kmean[d]*vmean[e]*hmask (rank-1/head)
                vrp = ppool.tile([128, 512], dt.float32, tag="rot", bufs=2,
                                 name=f"vrp{j}")
                nc.tensor.transpose(vrp[0:1, 0:128], stats2[:, 66 + j:67 + j],
                                    id32)
                vrow = const.tile([1, 128], dt.float32, tag=f"vrow{j}",
                                  name=f"vrow{j}")
                nc.scalar.mul(vrow, vrp[0:1, 0:128], -1.0 / N)
                vrowb = const.tile([128, 128], dt.float32, tag=f"vrowb{j}",
                                   name=f"vrowb{j}")
                nc.gpsimd.partition_broadcast(vrowb, vrow)
                mc0 = const.tile([128, 128], dt.float32, tag=f"mc0{j}",
                                 name=f"mc0{j}")
                nc.vector.tensor_tensor(
                    mc0, zsc[:, j:j + 1].to_broadcast((128, 128)), vrowb,
                    OP.mult)
                mc = const.tile([128, 128], dt.bfloat16, tag=f"mc{j}",
                                name=f"mc{j}")
                nc.vector.tensor_tensor(mc, mc0, hmask, OP.mult)
                mcorr.append(mc)

            # =========================== phase C ===========================
            stC = {}

            def emit_mmC(c):
                zps, aps, rps = [], [], []
                for j in range(2):
                    gsl = slice(c * 1024 + j * 512, c * 1024 + (j + 1) * 512)
                    zp = tpt2(f"zps{j}_{c}")
                    nc.tensor.matmul(zp, zblk[j], q1p[:, gsl],
                                     start=True, stop=True)
                    zps.append(zp)
                    ap = pj(f"aps{j}_{c}")
                    nc.tensor.matmul(ap, kvblk[j], qsp[:, gsl],
                                     start=True, stop=True)
                    aps.append(ap)
                for j in range(2):
                    gsl = slice(c * 1024 + j * 512, c * 1024 + (j + 1) * 512)
                    # rest = mcorr^T q1 + lepe (3 dcw taps), PSUM-accumulated
                    rp = pj(f"rps{j}_{c}")
                    nc.tensor.matmul(rp, mcorr[j], q1p[:, gsl],
                                     start=True, stop=False)
                    for tap in range(3):
                        nc.tensor.matmul(
                            rp, dcw[:, (tap * 2 + j) * 128:
                                    (tap * 2 + j + 1) * 128],
                            vT[j][:, c * CH + tap: c * CH + tap + 512],
                            start=False, stop=(tap == 2))
                    rps.append(rp)
                stC[c] = (zps, aps, rps)

            def emit_tailC(c):
                zps, aps, rps = stC.pop(c)
                ys = []
                for j in range(2):
                    gsl = slice(c * 1024 + j * 512, c * 1024 + (j + 1) * 512)
                    rz = wt("rz", dt.float32)
                    nc.vector.reciprocal_approx_fast(out=rz, in_=zps[j])
                    a1 = wt("a1")
                    nc.vector.scalar_tensor_tensor(
                        out=a1, in0=rz, scalar=1.0, in1=aps[j],
                        op0=OP.add, op1=OP.mult)
                    r2 = wt("r2")
                    nc.vector.tensor_add(r2, a1, rps[j])
                    yj = wt(f"y{j}")
                    nc.gpsimd.tensor_mul(yj, r2, o1p[:, gsl])
                    ys.append(yj)
                outp = [rot(f"outp{h}_{c}") for h in range(2)]
                for s in range(4):
                    osl = slice((s % 2) * 256, (s % 2) * 256 + 256)
                    nc.tensor.matmul(outp[s // 2][:, osl],
                                     ys[0][:, s * 128:(s + 1) * 128], wp[0],
                                     start=True, stop=False)
                    nc.tensor.matmul(outp[s // 2][:, osl],
                                     ys[1][:, s * 128:(s + 1) * 128], wp[1],
                                     start=False, stop=True)
                outsb = wt("outsb", dt.float32, cols=1024)
                nc.scalar.activation(outsb[:, 0:512], outp[0], AF.Copy)
                nc.scalar.activation(outsb[:, 512:1024], outp[1], AF.Copy)
                dsl = out_d[c * CH: (c + 1) * CH, :]
                nc.sync.dma_start(out=dsl.rearrange("(s t) o -> t s o", s=4),
                                  in_=outsb)

            for c in range(NCH):
                emit_mmC(c)
                if c > 0:
                    emit_tailC(c - 1)
            emit_tailC(NCH - 1)

            if dbg:
                nc.sync.dma_start(out=dbg32_d[:, :], in_=stats2)
                nc.sync.dma_start(out=dbg16_d[:, 0:1024], in_=q1p[:, 0:1024])
                nc.sync.dma_start(out=dbg16_d[:, 1024:2048], in_=qsp[:, 0:1024])
                nc.sync.dma_start(out=dbg16_d[:, 2048:3072], in_=o1p[:, 0:1024])
                nc.sync.dma_start(out=dbg16_d[:, 3072:4096],
                                  in_=vT[0][:, 0:1024])

    nc.compile()
    return nc


_NC_CACHE = {}


def _get_nc():
    key = (bool(os.environ.get("KERNEL_NOCC")),
           bool(os.environ.get("KERNEL_DBG")))
    if key not in _NC_CACHE:
        _NC_CACHE[key] = _build_nc()
    return _NC_CACHE[key]


def kernel(x, sin, cos, W_qkvo, b_qkvo, W_lepe, b_lepe, W_proj, b_proj):
    from concourse.bass_utils import run_bass_kernel_spmd

    per_core = _host_prep(x, sin, cos, W_qkvo, b_qkvo, W_lepe, b_lepe,
                          W_proj, b_proj)
    nc = _get_nc()
    # keep only the inputs that survived DCE in the compiled program
    import concourse.mybir as mybir
    expected = set()
    for alloc in nc.m.functions[0].allocations:
        if isinstance(alloc, mybir.MemoryLocationSet) and alloc.kind == "ExternalInput":
            expected.add(alloc.memorylocations[0].name)
    per_core = [{k: v for k, v in m.items() if k in expected} for m in per_core]
    res = run_bass_kernel_spmd(nc, per_core, core_ids=list(range(NCORES)),
                               trace=bool(os.environ.get("KERNEL_TRACE")))
    if os.environ.get("KERNEL_TRACE"):
        kernel.last_exec_time_ns = res.exec_time_ns
        kernel.last_results = res
    full = np.zeros((B, N, INTERNAL), np.float32)
    for c in range(NCORES):
        b = c // 2
        t0 = (c % 2) * T
        full[b, t0:t0 + T] = res.results[c]["out"]
    return full


# ---------------------------------------------------------- numpy selftest
# numpy emulation of the exact device pipeline (fp32), validates the
# decomposition (run with KERNEL_SELFTEST=1).

def _numpy_pipeline(per_core_inputs):
    cores = []
    for c in range(NCORES):
        d = per_core_inputs[c]
        xct = d["xct"].astype(np.float32)          # [256, TH]
        srep = d["srep"].astype(np.float32)
        crep = d["crep"].astype(np.float32)
        wq = d["wq"].astype(np.float32)
        wkv = d["wkv"].astype(np.float32)
        wo = d["wo"].astype(np.float32)
        R = d["rblk"].astype(np.float32)

        x_in = xct[:, 1:T + 1]                     # [256, T]
        qT = wq.T @ x_in                           # [256, T]
        kT = wkv[:, 0:256].T @ x_in
        vT_m = wkv[:, 256:512].T @ x_in
        oT = wo.T @ x_in
        vhl = wkv[:, 256:512].T @ xct[:, 0:1]
        vhr = wkv[:, 256:512].T @ xct[:, TH - 1:TH]
        vT = np.concatenate([vhl, vT_m, vhr], axis=1)      # [256, TH]

        def elu1(t):
            return np.minimum(np.exp(t), 1.0) + np.maximum(t, 0.0)

        q1 = elu1(qT)
        k1 = elu1(kT)

        ks = np.zeros_like(k1)
        qs = np.zeros_like(q1)
        for j in range(2):
            sl = slice(128 * j, 128 * (j + 1))
            ks[sl] = k1[sl] * crep + (R.T @ k1[sl]) * srep
            qs[sl] = q1[sl] * crep + (R.T @ q1[sl]) * srep

        gram = np.zeros((128, 256), np.float32)
        for j in range(2):
            gram[:, 128 * j:128 * (j + 1)] = (
                ks[128 * j:128 * (j + 1)] @ vT[128 * j:128 * (j + 1), 1:T + 1].T)
        ksum = k1.sum(axis=1)                      # [256]
        vsum = vT[:, 1:T + 1].sum(axis=1)
        cores.append(dict(d=d, q1=q1, qs=qs, oT=oT, vT=vT, gram=gram,
                          ksum=ksum, vsum=vsum))

    for pair in range(4):
        a, b2 = cores[2 * pair], cores[2 * pair + 1]
        gram = a["gram"] + b2["gram"]
        ksum = a["ksum"] + b2["ksum"]
        vsum = a["vsum"] + b2["vsum"]
        for cc in (a, b2):
            cc["gram_r"], cc["ksum_r"], cc["vsum_r"] = gram, ksum, vsum

    outs = []
    for c in range(NCORES):
        st = cores[c]
        d = st["d"]
        q1, qs, oT, vT = st["q1"], st["qs"], st["oT"], st["vT"]
        hmask = d["hmask"].astype(np.float32)
        wl6 = d["wl6"].astype(np.float32)
        wp = d["wp"].astype(np.float32)
        gram, ksum, vsum = st["gram_r"], st["ksum_r"], st["vsum_r"]

        res = np.zeros((256, T), np.float32)
        for j in range(2):
            sl = slice(128 * j, 128 * (j + 1))
            zsc = (SCALE / N) * ksum[sl]                     # [128]
            zblk = zsc[:, None] * hmask                      # [128,128]
            zps = zblk.T @ q1[sl]                            # [128, T]
            kvblk = np.zeros((128, 128), np.float32)
            for aa in range(4):
                s2 = slice(32 * aa, 32 * (aa + 1))
                kvblk[s2, s2] = KSC * gram[s2, 128 * j + 32 * aa:
                                           128 * j + 32 * (aa + 1)]
            aps = kvblk.T @ qs[sl]
            a1 = (1.0 / zps + 1.0) * aps
            lepe = (wl6[:, 3 * j:3 * j + 1] * vT[sl, 0:T]
                    + wl6[:, 3 * j + 1:3 * j + 2] * vT[sl, 1:T + 1]
                    + wl6[:, 3 * j + 2:3 * j + 3] * vT[sl, 2:T + 2])
            c1 = zps * (-vsum[sl] / N)[:, None] + lepe
            res[sl] = a1 + c1
        y = res * oT
        outs.append((y.T @ wp).astype(np.float32))

    full = np.zeros((B, N, 256), np.float32)
    for c in range(NCORES):
        b = c // 2
        t0 = (c % 2) * T
        full[b, t0:t0 + T] = outs[c]
    return full


if __name__ == "__main__" and os.environ.get("KERNEL_SELFTEST"):
    sys.path.insert(0, os.path.dirname(os.path.abspath(__file__)))
    import reference
    inputs = {k: np.asarray(v) for k, v in reference.setup_inputs().items()}
    expected = np.asarray(reference.reference(**inputs))
    per_core = _host_prep(**inputs)
    got = _numpy_pipeline(per_core)
    rel = np.linalg.norm(got - expected) / np.linalg.norm(expected)
    print("selftest rel err:", rel, "max abs:", np.abs(got - expected).max())

if __name__ == "__main__" and os.environ.get("KERNEL_BUILD"):
    nc = _build_nc()
    print("build OK")


# revision 29
# speedup vs baseline: 1.2099x; 1.0596x over previous
"""Trainium2 Bass kernel for nn_MultiHeadMALAAttention.

Sharding: 8 cores; core c handles batch b = c//2, token half h = c%2
(tokens [h*4096, (h+1)*4096) of N=8192).  Stats (ksum, vsum, kv gram)
need full-N reductions -> pairwise AllReduce between the two cores of a
batch, replica groups [[0,1],[2,3],[4,5],[6,7]].

Structure (per core), ordered so the AllReduce hides under compute:
  dummy 32B AllReduce first (absorbs the one-time CC barrier under the
  initial weight/x DMAs)
  phase A: k,v projections -> elu(k)+1 -> rope(k) -> PE transposes ->
           kv gram accumulation in PSUM; ksum/vsum ride accum_out.
  stats pack ([128,68]) -> AllReduce
  phase B: q,o projections -> elu(q)+1 -> rope(q) -> qs  (runs during AR)
  phase C: z = zblk^T q1; attn = kvblk^T qs; then everything else on DVE:
           res = (1+1/z)*attn - z*vmean + lepe(v), y = res*o, out proj.

Key fusions vs a naive pipeline: elu+1 = min(exp(x),1)+relu(x) (2 ACT +
1 stt); the (1+1/z) factor commutes past the per-head kv matmul so
rope(q) is stats-independent; -z*vmean rides the z PSUM as one stt; the
depthwise conv (LEPE) is 3 stt ops with per-partition channel weights.
"""

import os
import sys

sys.path.insert(0, "/opt/trn_rl_repo")

import numpy as np
import ml_dtypes

B, N, DIM, H, HD = 4, 8192, 256, 8, 32
INTERNAL = H * HD  # 256
SCALE = HD ** -0.5
NCORES = 8
T = N // 2          # tokens per core
TH = T + 2          # with 1-token halo each side
CH = 512            # chunk tokens
NCH = T // CH       # chunks per core
KSC = SCALE / N     # kv_state scale (s^2)

BF16 = ml_dtypes.bfloat16


# ---------------------------------------------------------------- host prep

def _host_prep(x, sin, cos, W_qkvo, b_qkvo, W_lepe, b_lepe, W_proj, b_proj):
    """Build per-core input dicts (all device tensors)."""
    assert not (np.any(b_qkvo) or np.any(b_lepe) or np.any(b_proj)), \
        "bias path not implemented (setup_inputs uses zero biases)"

    WT = W_qkvo.T.astype(np.float32)          # [DIM, 1024] = lhsT layout
    wq = WT[:, 0:256].astype(BF16)
    wkv = WT[:, 256:768].astype(BF16)          # k cols 0:256, v cols 256:512
    wo = WT[:, 768:1024].astype(BF16)
    wp = W_proj.T.astype(np.float32).astype(BF16)   # [DIM, 256] rhs layout
    wl = W_lepe[:, 0, :].astype(np.float32)    # [256, 3]
    # per-partition lepe weights: tile j cols 3j..3j+3
    wl6 = np.zeros((128, 6), np.float32)
    wl6[:, 0:3] = wl[0:128]
    wl6[:, 3:6] = wl[128:256]

    # diag conv weights: block (tap j, tile m) = diag(wl[128m:128(m+1), j])
    dcw = np.zeros((128, 6, 128), np.float32)
    for j in range(3):
        for m in range(2):
            np.fill_diagonal(dcw[:, j * 2 + m, :], wl[128 * m:128 * (m + 1), j])
    dcw = dcw.reshape(128, 768).astype(BF16)

    # rotate-every-two matrix as lhsT: rot = R.T @ x ; R[k, m] = coeff of
    # chan k in rot-chan m:  rot[2i] = -x[2i+1], rot[2i+1] = x[2i]
    R = np.zeros((128, 128), np.float32)
    for i in range(64):
        R[2 * i + 1, 2 * i] = -1.0
        R[2 * i, 2 * i + 1] = 1.0
    R = R.astype(BF16)

    hmask = np.zeros((128, 128), np.float32)
    for hh in range(4):
        hmask[32 * hh:32 * (hh + 1), 32 * hh:32 * (hh + 1)] = 1.0
    hmask = hmask.astype(BF16)

    ident16 = np.eye(128, dtype=np.float32).astype(BF16)
    ident32 = np.eye(128, dtype=np.float32)

    xf = np.asarray(x, np.float32)
    sinf = np.asarray(sin, np.float32)
    cosf = np.asarray(cos, np.float32)

    per_core = []
    for c in range(NCORES):
        b = c // 2
        t0 = (c % 2) * T
        # x channel-major with halo [256, TH]
        xpad = np.zeros((TH, DIM), np.float32)
        lo, hi = t0 - 1, t0 + T + 1
        slo, shi = max(lo, 0), min(hi, N)
        xpad[slo - lo: slo - lo + (shi - slo)] = xf[b, slo:shi]
        xct = np.ascontiguousarray(xpad.T).astype(BF16)          # [256, TH]

        srep = np.tile(sinf[t0:t0 + T].T, (4, 1)).astype(BF16)   # [128, T]
        crep = np.tile(cosf[t0:t0 + T].T, (4, 1)).astype(BF16)   # [128, T]

        per_core.append({
            "xct": xct, "srep": np.ascontiguousarray(srep),
            "crep": np.ascontiguousarray(crep),
            "wq": np.ascontiguousarray(wq), "wkv": np.ascontiguousarray(wkv),
            "wo": np.ascontiguousarray(wo), "wp": np.ascontiguousarray(wp),
            "rblk": R, "hmask": hmask, "ident16": ident16, "wl6": wl6,
            "ident32": ident32, "dcw": dcw,
        })
    return per_core


# ------------------------------------------------------------ device kernel

def _build_nc():
    from concourse import bacc
    import concourse.mybir as mybir
    import concourse.tile as tile

    dt = mybir.dt
    AF = mybir.ActivationFunctionType
    OP = mybir.AluOpType

    nocc = bool(os.environ.get("KERNEL_NOCC"))

    nc = bacc.Bacc(None, target_bir_lowering=False)

    # ---- I/O
    xct_d = nc.dram_tensor("xct", [256, TH], dt.bfloat16, kind="ExternalInput")
    srep_d = nc.dram_tensor("srep", [128, T], dt.bfloat16, kind="ExternalInput")
    crep_d = nc.dram_tensor("crep", [128, T], dt.bfloat16, kind="ExternalInput")
    wq_d = nc.dram_tensor("wq", [256, 256], dt.bfloat16, kind="ExternalInput")
    wkv_d = nc.dram_tensor("wkv", [256, 512], dt.bfloat16, kind="ExternalInput")
    wo_d = nc.dram_tensor("wo", [256, 256], dt.bfloat16, kind="ExternalInput")
    wp_d = nc.dram_tensor("wp", [256, 256], dt.bfloat16, kind="ExternalInput")
    rblk_d = nc.dram_tensor("rblk", [128, 128], dt.bfloat16, kind="ExternalInput")
    hmask_d = nc.dram_tensor("hmask", [128, 128], dt.bfloat16, kind="ExternalInput")
    id16_d = nc.dram_tensor("ident16", [128, 128], dt.bfloat16, kind="ExternalInput")
    id32_d = nc.dram_tensor("ident32", [128, 128], dt.float32, kind="ExternalInput")
    dcw_d = nc.dram_tensor("dcw", [128, 768], dt.bfloat16, kind="ExternalInput")
    out_d = nc.dram_tensor("out", [T, 256], dt.float32, kind="ExternalOutput")
    dbg = bool(os.environ.get("KERNEL_DBG"))
    if dbg:
        dbg16_d = nc.dram_tensor("dbg16", [128, 4096], dt.bfloat16,
                                 kind="ExternalOutput")
        dbg32_d = nc.dram_tensor("dbg32", [128, 68], dt.float32,
                                 kind="ExternalOutput")

    RG = [[0, 1], [2, 3], [4, 5], [6, 7]]

    with tile.TileContext(nc) as tc:
        with (
            tc.tile_pool(name="const", bufs=1) as const,
            tc.tile_pool(name="work", bufs=2) as work,
            tc.tile_pool(name="psum", bufs=1, space="PSUM") as ppool,
            tc.tile_pool(name="dram", bufs=1, space="DRAM") as dpool,
        ):
            # ---- dummy collective: absorb the one-time CC barrier early
            if not nocc:
                dum = const.tile([1, 8], dt.float32, tag="dum", name="dum")
                nc.vector.memset(dum, 0.0)
                ccd_i = dpool.tile([1, 8], dt.float32, tag="ccdi", name="ccdi")
                ccd_o = dpool.tile([1, 8], dt.float32, tag="ccdo", name="ccdo")
                nc.gpsimd.dma_start(out=ccd_i[:, :], in_=dum)
                nc.gpsimd.collective_compute(
                    "AllReduce", OP.add, replica_groups=RG,
                    ins=[ccd_i[:, :]], outs=[ccd_o[:, :]])

            # ---- consts
            def load(tname, dten, shape, dtype=dt.bfloat16):
                t_ = const.tile(shape, dtype, tag=tname, name=tname)
                nc.sync.dma_start(out=t_, in_=dten[:, :])
                return t_

            wq = [const.tile([128, 256], dt.bfloat16, tag=f"wq{k}", name=f"wq{k}")
                  for k in range(2)]
            wkv = [const.tile([128, 512], dt.bfloat16, tag=f"wkv{k}", name=f"wkv{k}")
                   for k in range(2)]
            wo = [const.tile([128, 256], dt.bfloat16, tag=f"wo{k}", name=f"wo{k}")
                  for k in range(2)]
            wp = [const.tile([128, 256], dt.bfloat16, tag=f"wp{k}", name=f"wp{k}")
                  for k in range(2)]
            # load order: phase A's critical path first (wkv, x chunk 0,
            # rope consts), then the rest
            xct = [const.tile([128, TH], dt.bfloat16, tag=f"xct{k}",
                              name=f"xct{k}") for k in range(2)]
            srep = const.tile([128, T], dt.bfloat16, tag="srep", name="srep")
            crep = const.tile([128, T], dt.bfloat16, tag="crep", name="crep")
            XSPL = [0, 1026, 2050, 3074, TH]
            for k in range(2):
                nc.sync.dma_start(out=wkv[k], in_=wkv_d[128 * k:128 * (k + 1), :])
            for k in range(2):
                nc.sync.dma_start(out=xct[k][:, 0:1026], in_=xct_d[128 * k:128 * (k + 1), 0:1026])
            rblk = load("rblk", rblk_d, [128, 128])
            id16 = load("id16", id16_d, [128, 128])
            nc.sync.dma_start(out=srep[:, 0:2048], in_=srep_d[:, 0:2048])
            nc.sync.dma_start(out=crep[:, 0:2048], in_=crep_d[:, 0:2048])
            for k in range(2):
                for si in range(1, 4):
                    cs = slice(XSPL[si], XSPL[si + 1])
                    nc.sync.dma_start(out=xct[k][:, cs],
                                      in_=xct_d[128 * k:128 * (k + 1), cs])
            nc.sync.dma_start(out=srep[:, 2048:T], in_=srep_d[:, 2048:T])
            nc.sync.dma_start(out=crep[:, 2048:T], in_=crep_d[:, 2048:T])
            for k in range(2):
                sl = slice(128 * k, 128 * (k + 1))
                nc.sync.dma_start(out=wq[k], in_=wq_d[sl, :])
                nc.sync.dma_start(out=wo[k], in_=wo_d[sl, :])
                nc.sync.dma_start(out=wp[k], in_=wp_d[sl, :])
            hmask = load("hmask", hmask_d, [128, 128])
            id32 = load("id32", id32_d, [128, 128], dt.float32)
            dcw = load("dcw", dcw_d, [128, 768])

            # persistent activations (paired layout: col c*1024 + j*512 + t)
            q1p = const.tile([128, 2 * T], dt.bfloat16, tag="q1p", name="q1p")
            qsp = const.tile([128, 2 * T], dt.bfloat16, tag="qsp", name="qsp")
            o1p = const.tile([128, 2 * T], dt.bfloat16, tag="o1p", name="o1p")
            vT = [const.tile([128, TH], dt.bfloat16, tag=f"vT{j}", name=f"vT{j}")
                  for j in range(2)]
            kpart = const.tile([128, 16], dt.float32, tag="kpart", name="kpart")
            vpart = const.tile([128, 16], dt.float32, tag="vpart", name="vpart")
            stats = const.tile([128, 68], dt.float32, tag="stats", name="stats")
            stats2 = const.tile([128, 68], dt.float32, tag="stats2", name="stats2")

            # psum: "pj" 4 banks, "rot" 2 banks, "tp" 2x bf16 (1 bank), "acc" 1
            def pj(nm):
                return ppool.tile([128, 512], dt.float32, tag="pj", bufs=3,
                                  name=nm)

            def rot(nm):
                return ppool.tile([128, 512], dt.float32, tag="rot", bufs=2,
                                  name=nm)

            def tpt(nm):
                return ppool.tile([128, 512], dt.bfloat16, tag="tp", bufs=2,
                                  name=nm)

            def tpt2(nm):
                return ppool.tile([128, 512], dt.float32, tag="tp", bufs=2,
                                  name=nm)

            acc = ppool.tile([128, 512], dt.float32, tag="acc", bufs=1,
                             name="acc")
            gram = acc[:, 0:256]

            def wt(nm, dtype=dt.bfloat16, cols=512, bufs=2):
                return work.tile([128, cols], dtype, tag=nm, bufs=bufs, name=nm)

            # =========================== phase A (k, v) ====================
            stA = {}

            def emit_projA(c):
                xs = [x_[:, 1 + c * CH: 1 + (c + 1) * CH] for x_ in xct]
                kps, vps = [], []
                for j in range(2):
                    p = pj(f"kps{j}_{c}")
                    nc.tensor.matmul(p, wkv[0][:, 128 * j:128 * (j + 1)], xs[0],
                                     start=True, stop=False)
                    nc.tensor.matmul(p, wkv[1][:, 128 * j:128 * (j + 1)], xs[1],
                                     start=False, stop=True)
                    kps.append(p)
                for j in range(2):
                    p = pj(f"vps{j}_{c}")
                    vsl = slice(256 + 128 * j, 256 + 128 * (j + 1))
                    nc.tensor.matmul(p, wkv[0][:, vsl], xs[0],
                                     start=True, stop=False)
                    nc.tensor.matmul(p, wkv[1][:, vsl], xs[1],
                                     start=False, stop=True)
                    vps.append(p)
                stA[c] = (kps, vps)

            def emit_tailA(c):
                kps, vps = stA.pop(c)
                ssl = srep[:, c * CH:(c + 1) * CH]
                csl = crep[:, c * CH:(c + 1) * CH]
                k1s = []
                for j in range(2):
                    # elu(k)+1 = min(exp(k),1) + relu(k); ksum rides the stt
                    e = wt("e")
                    nc.scalar.activation(e, kps[j], AF.Exp)
                    r = wt("r")
                    nc.scalar.activation(r, kps[j], AF.Relu)
                    k1 = wt("k1")
                    nc.vector.scalar_tensor_tensor(
                        out=k1, in0=e, scalar=1.0, in1=r,
                        op0=OP.min, op1=OP.add,
                        accum_out=kpart[:, 8 * j + c: 8 * j + c + 1])
                    k1s.append(k1)
                ks = []
                for j in range(2):
                    k1 = k1s[j]
                    # v evac (vsum rides accum)
                    nc.scalar.activation(
                        vT[j][:, 1 + c * CH: 1 + (c + 1) * CH], vps[j],
                        AF.Copy, accum_out=vpart[:, 8 * j + c: 8 * j + c + 1])
                    # rope(k)
                    rk = rot(f"rk{j}_{c}")
                    nc.tensor.matmul(rk, rblk, k1, start=True, stop=True)
                    m2 = wt("m2")
                    nc.vector.tensor_mul(m2, rk, ssl)
                    m1 = wt("m1")
                    # early chunks avoid Pool: the dummy-CC barrier parks the
                    # GpSimd queue for ~13us at kernel start
                    if c < 3:
                        nc.vector.tensor_mul(m1, k1, csl)
                    else:
                        nc.gpsimd.tensor_mul(m1, k1, csl)
                    ksj = wt(f"ks{j}")
                    nc.vector.tensor_add(ksj, m1, m2)
                    ks.append(ksj)
                # transpose + gram accumulation
                for s in range(4):
                    ktp = tpt(f"ktp{c}_{s}")
                    nc.tensor.transpose(ktp[:, 0:128],
                                        ks[0][:, s * 128:(s + 1) * 128], id16)
                    nc.tensor.transpose(ktp[:, 128:256],
                                        ks[1][:, s * 128:(s + 1) * 128], id16)
                    vcol = 1 + c * CH + s * 128
                    nc.tensor.transpose(ktp[:, 256:384],
                                        vT[0][:, vcol:vcol + 128], id16)
                    nc.tensor.transpose(ktp[:, 384:512],
                                        vT[1][:, vcol:vcol + 128], id16)
                    kvtok = wt("kvtok")
                    if s % 2 == 0:
                        nc.vector.tensor_copy(kvtok, ktp)
                    else:
                        nc.scalar.activation(kvtok, ktp, AF.Copy)
                    # PSUM start=True resets at coarser-than-region
                    # granularity: only the very first matmul into this bank
                    # may carry start=True; the second column group
                    # accumulates onto the zeros that reset left behind.
                    first = (c == 0 and s == 0)
                    last = (c == NCH - 1 and s == 3)
                    nc.tensor.matmul(gram[:, 0:128], kvtok[:, 0:128],
                                     kvtok[:, 256:384], start=first, stop=False)
                    nc.tensor.matmul(gram[:, 128:256], kvtok[:, 128:256],
                                     kvtok[:, 384:512], start=False, stop=last)

            for c in range(NCH):
                emit_projA(c)
                if c > 0:
                    emit_tailA(c - 1)
            emit_tailA(NCH - 1)

            # ================= stats pack + AllReduce ======================
            # gram diag head-blocks -> stats[:, 32j:32j+32]
            for j in range(2):
                for a in range(4):
                    psl = slice(32 * a, 32 * (a + 1))
                    nc.vector.tensor_copy(
                        stats[psl, 32 * j:32 * (j + 1)],
                        gram[psl, 128 * j + 32 * a:128 * j + 32 * (a + 1)])
            nc.vector.tensor_reduce(stats[:, 64:65], kpart[:, 0:8],
                                    axis=mybir.AxisListType.X, op=OP.add)
            nc.vector.tensor_reduce(stats[:, 65:66], kpart[:, 8:16],
                                    axis=mybir.AxisListType.X, op=OP.add)
            nc.vector.tensor_reduce(stats[:, 66:67], vpart[:, 0:8],
                                    axis=mybir.AxisListType.X, op=OP.add)
            nc.vector.tensor_reduce(stats[:, 67:68], vpart[:, 8:16],
                                    axis=mybir.AxisListType.X, op=OP.add)

            if nocc:
                nc.vector.tensor_scalar_mul(stats2, stats, 1.0)
            else:
                ccin = dpool.tile([128, 68], dt.float32, tag="ccin", name="ccin")
                ccout = dpool.tile([128, 68], dt.float32, tag="ccout",
                                   name="ccout")
                nc.gpsimd.dma_start(out=ccin[:, :], in_=stats)
                nc.gpsimd.collective_compute(
                    "AllReduce", OP.add, replica_groups=RG,
                    ins=[ccin[:, :]], outs=[ccout[:, :]])
                nc.gpsimd.dma_start(out=stats2, in_=ccout[:, :])

            # =========================== phase B (q, o) ====================
            # halo v columns (tokens t0-1 and t0+T) for the conv; single
            # accumulation group (one start, one stop).  Lives in the second
            # half-bank of acc: PSUM start=True resets a 1 KB half-bank, so
            # it must not share a half-bank with the live gram.
            hal = acc[:, 384:400]
            for j in range(2):
                vsl = slice(256 + 128 * j, 256 + 128 * (j + 1))
                cl = slice(j * 4, j * 4 + 1)
                cr = slice(j * 4 + 2, j * 4 + 3)
                nc.tensor.matmul(hal[:, cl], wkv[0][:, vsl], xct[0][:, 0:1],
                                 start=(j == 0), stop=False)
                nc.tensor.matmul(hal[:, cl], wkv[1][:, vsl], xct[1][:, 0:1],
                                 start=False, stop=False)
                nc.tensor.matmul(hal[:, cr], wkv[0][:, vsl],
                                 xct[0][:, TH - 1:TH], start=False, stop=False)
                nc.tensor.matmul(hal[:, cr], wkv[1][:, vsl],
                                 xct[1][:, TH - 1:TH], start=False,
                                 stop=(j == 1))
            for j in range(2):
                nc.scalar.activation(vT[j][:, 0:1],
                                     hal[:, j * 4:j * 4 + 1], AF.Copy)
                nc.scalar.activation(vT[j][:, TH - 1:TH],
                                     hal[:, j * 4 + 2:j * 4 + 3], AF.Copy)

            stB = {}

            def emit_projB(c):
                xs = [x_[:, 1 + c * CH: 1 + (c + 1) * CH] for x_ in xct]
                qps, ops_ = [], []
                for j in range(2):
                    p = pj(f"qps{j}_{c}")
                    nc.tensor.matmul(p, wq[0][:, 128 * j:128 * (j + 1)], xs[0],
                                     start=True, stop=False)
                    nc.tensor.matmul(p, wq[1][:, 128 * j:128 * (j + 1)], xs[1],
                                     start=False, stop=True)
                    qps.append(p)
                for j in range(2):
                    p = pj(f"ops{j}_{c}")
                    nc.tensor.matmul(p, wo[0][:, 128 * j:128 * (j + 1)], xs[0],
                                     start=True, stop=False)
                    nc.tensor.matmul(p, wo[1][:, 128 * j:128 * (j + 1)], xs[1],
                                     start=False, stop=True)
                    ops_.append(p)
                stB[c] = (qps, ops_)

            def emit_tailB(c):
                qps, ops_ = stB.pop(c)
                ssl = srep[:, c * CH:(c + 1) * CH]
                csl = crep[:, c * CH:(c + 1) * CH]
                for j in range(2):
                    gsl = slice(c * 1024 + j * 512, c * 1024 + (j + 1) * 512)
                    e = wt("e")
                    nc.scalar.activation(e, qps[j], AF.Exp)
                    r = wt("r")
                    nc.scalar.activation(r, qps[j], AF.Relu)
                    nc.vector.scalar_tensor_tensor(
                        out=q1p[:, gsl], in0=e, scalar=1.0, in1=r,
                        op0=OP.min, op1=OP.add)
                    nc.vector.tensor_copy(o1p[:, gsl], ops_[j])
                    rq = rot(f"rq{j}_{c}")
                    nc.tensor.matmul(rq, rblk, q1p[:, gsl],
                                     start=True, stop=True)
                    m2 = wt("m2")
                    nc.vector.tensor_mul(m2, rq, ssl)
                    m1 = wt("m1")
                    nc.gpsimd.tensor_mul(m1, q1p[:, gsl], csl)
                    nc.vector.tensor_add(qsp[:, gsl], m1, m2)

            for c in range(NCH):
                emit_projB(c)
                if c > 0:
                    emit_tailB(c - 1)
            emit_tailB(NCH - 1)

            # ================= phase C consts (post-AR) ====================
            zsc = const.tile([128, 2], dt.float32, tag="zsc", name="zsc")
            nc.scalar.mul(zsc, stats2[:, 64:66], SCALE / N)
            zblk, kvblk, mcorr = [], [], []
            for j in range(2):
                zb = const.tile([128, 128], dt.bfloat16, tag=f"zb{j}",
                                name=f"zb{j}")
                nc.vector.tensor_tensor(
                    zb, zsc[:, j:j + 1].to_broadcast((128, 128)), hmask,
                    OP.mult)
                zblk.append(zb)
                kvb = const.tile([128, 128], dt.bfloat16, tag=f"kvb{j}",
                                 name=f"kvb{j}")
                nc.vector.memset(kvb, 0.0)
                for a in range(4):
                    psl = slice(32 * a, 32 * (a + 1))
                    nc.scalar.mul(kvb[psl, psl],
                                  stats2[psl, 32 * j:32 * (j + 1)], KSC)
                kvblk.append(kvb)

            def emit_mcorr():
                # mcorr[d,e] = -SCALE*kmean[d]*vmean[e]*hmask (rank-1/head)
                for j in range(2):
                    vrp = ppool.tile([128, 512], dt.float32, tag="rot",
                                     bufs=2, name=f"vrp{j}")
                    nc.tensor.transpose(vrp[0:1, 0:128],
                                        stats2[:, 66 + j:67 + j], id32)
                    vrow = const.tile([1, 128], dt.float32, tag=f"vrow{j}",
                                      name=f"vrow{j}")
                    nc.scalar.mul(vrow, vrp[0:1, 0:128], -1.0 / N)
                    vrowb = const.tile([128, 128], dt.float32,
                                       tag=f"vrowb{j}", name=f"vrowb{j}")
                    nc.gpsimd.partition_broadcast(vrowb, vrow)
                    mc0 = const.tile([128, 128], dt.float32, tag=f"mc0{j}",
                                     name=f"mc0{j}")
                    nc.vector.tensor_tensor(
                        mc0, zsc[:, j:j + 1].to_broadcast((128, 128)), vrowb,
                        OP.mult)
                    mc = const.tile([128, 128], dt.bfloat16, tag=f"mc{j}",
                                    name=f"mc{j}")
                    nc.vector.tensor_tensor(mc, mc0, hmask, OP.mult)
                    mcorr.append(mc)

            # =========================== phase C ===========================
            stC = {}

            def emit_za(c):
                zps, aps = [], []
                for j in range(2):
                    gsl = slice(c * 1024 + j * 512, c * 1024 + (j + 1) * 512)
                    if j == 0:
                        zp = tpt2(f"zps{j}_{c}")
                    else:
                        zp = ppool.tile([128, 512], dt.float32, tag="acc",
                                        bufs=1, name=f"zps{j}_{c}")
                    nc.tensor.matmul(zp, zblk[j], q1p[:, gsl],
                                     start=True, stop=True)
                    zps.append(zp)
                    ap = pj(f"aps{j}_{c}")
                    nc.tensor.matmul(ap, kvblk[j], qsp[:, gsl],
                                     start=True, stop=True)
                    aps.append(ap)
                stC[c] = (zps, aps, None)

            def emit_rest(c):
                zps, aps, _ = stC[c]
                rps = []
                for j in range(2):
                    gsl = slice(c * 1024 + j * 512, c * 1024 + (j + 1) * 512)
                    # rest = mcorr^T q1 + lepe (3 dcw taps), PSUM-accumulated
                    rp = pj(f"rps{j}_{c}")
                    nc.tensor.matmul(rp, mcorr[j], q1p[:, gsl],
                                     start=True, stop=False)
                    for tap in range(3):
                        nc.tensor.matmul(
                            rp, dcw[:, (tap * 2 + j) * 128:
                                    (tap * 2 + j + 1) * 128],
                            vT[j][:, c * CH + tap: c * CH + tap + 512],
                            start=False, stop=(tap == 2))
                    rps.append(rp)
                stC[c] = (zps, aps, rps)

            def emit_tailC(c):
                zps, aps, rps = stC.pop(c)
                ys = []
                for j in range(2):
                    gsl = slice(c * 1024 + j * 512, c * 1024 + (j + 1) * 512)
                    rz = wt("rz", dt.float32)
                    nc.vector.reciprocal_approx_fast(out=rz, in_=zps[j])
                    a1 = wt("a1")
                    nc.vector.scalar_tensor_tensor(
                        out=a1, in0=rz, scalar=1.0, in1=aps[j],
                        op0=OP.add, op1=OP.mult)
                    r2 = wt("r2")
                    nc.vector.tensor_add(r2, a1, rps[j])
                    yj = wt(f"y{j}")
                    nc.gpsimd.tensor_mul(yj, r2, o1p[:, gsl])
                    ys.append(yj)
                outp = [rot(f"outp{h}_{c}") for h in range(2)]
                for s in range(4):
                    osl = slice((s % 2) * 256, (s % 2) * 256 + 256)
                    nc.tensor.matmul(outp[s // 2][:, osl],
                                     ys[0][:, s * 128:(s + 1) * 128], wp[0],
                                     start=True, stop=False)
                    nc.tensor.matmul(outp[s // 2][:, osl],
                                     ys[1][:, s * 128:(s + 1) * 128], wp[1],
                                     start=False, stop=True)
                outsb = wt("outsb", dt.float32, cols=1024)
                for h in range(2):
                    nc.scalar.activation(outsb[:, h * 512:(h + 1) * 512],
                                         outp[h], AF.Copy)
                    dsl = out_d[c * CH + h * 256: c * CH + (h + 1) * 256, :]
                    nc.sync.dma_start(
                        out=dsl.rearrange("(s t) o -> t s o", s=2),
                        in_=outsb[:, h * 512:(h + 1) * 512])

            emit_za(0)
            emit_mcorr()
            emit_rest(0)
            for c in range(1, NCH):
                emit_za(c)
                emit_rest(c)
                emit_tailC(c - 1)
            emit_tailC(NCH - 1)

            if dbg:
                nc.sync.dma_start(out=dbg32_d[:, :], in_=stats2)
                nc.sync.dma_start(out=dbg16_d[:, 0:1024], in_=q1p[:, 0:1024])
                nc.sync.dma_start(out=dbg16_d[:, 1024:2048], in_=qsp[:, 0:1024])
                nc.sync.dma_start(out=dbg16_d[:, 2048:3072], in_=o1p[:, 0:1024])
                nc.sync.dma_start(out=dbg16_d[:, 3072:4096],
                                  in_=vT[0][:, 0:1024])

    nc.compile()
    return nc


_NC_CACHE = {}


def _get_nc():
    key = (bool(os.environ.get("KERNEL_NOCC")),
           bool(os.environ.get("KERNEL_DBG")))
    if key not in _NC_CACHE:
        _NC_CACHE[key] = _build_nc()
    return _NC_CACHE[key]


def kernel(x, sin, cos, W_qkvo, b_qkvo, W_lepe, b_lepe, W_proj, b_proj):
    from concourse.bass_utils import run_bass_kernel_spmd

    per_core = _host_prep(x, sin, cos, W_qkvo, b_qkvo, W_lepe, b_lepe,
                          W_proj, b_proj)
    nc = _get_nc()
    # keep only the inputs that survived DCE in the compiled program
    import concourse.mybir as mybir
    expected = set()
    for alloc in nc.m.functions[0].allocations:
        if isinstance(alloc, mybir.MemoryLocationSet) and alloc.kind == "ExternalInput":
            expected.add(alloc.memorylocations[0].name)
    per_core = [{k: v for k, v in m.items() if k in expected} for m in per_core]
    res = run_bass_kernel_spmd(nc, per_core, core_ids=list(range(NCORES)),
                               trace=bool(os.environ.get("KERNEL_TRACE")))
    if os.environ.get("KERNEL_TRACE"):
        kernel.last_exec_time_ns = res.exec_time_ns
        kernel.last_results = res
    full = np.zeros((B, N, INTERNAL), np.float32)
    for c in range(NCORES):
        b = c // 2
        t0 = (c % 2) * T
        full[b, t0:t0 + T] = res.results[c]["out"]
    return full


# ---------------------------------------------------------- numpy selftest
# numpy emulation of the exact device pipeline (fp32), validates the
# decomposition (run with KERNEL_SELFTEST=1).

def _numpy_pipeline(per_core_inputs):
    cores = []
    for c in range(NCORES):
        d = per_core_inputs[c]
        xct = d["xct"].astype(np.float32)          # [256, TH]
        srep = d["srep"].astype(np.float32)
        crep = d["crep"].astype(np.float32)
        wq = d["wq"].astype(np.float32)
        wkv = d["wkv"].astype(np.float32)
        wo = d["wo"].astype(np.float32)
        R = d["rblk"].astype(np.float32)

        x_in = xct[:, 1:T + 1]                     # [256, T]
        qT = wq.T @ x_in                           # [256, T]
        kT = wkv[:, 0:256].T @ x_in
        vT_m = wkv[:, 256:512].T @ x_in
        oT = wo.T @ x_in
        vhl = wkv[:, 256:512].T @ xct[:, 0:1]
        vhr = wkv[:, 256:512].T @ xct[:, TH - 1:TH]
        vT = np.concatenate([vhl, vT_m, vhr], axis=1)      # [256, TH]

        def elu1(t):
            return np.minimum(np.exp(t), 1.0) + np.maximum(t, 0.0)

        q1 = elu1(qT)
        k1 = elu1(kT)

        ks = np.zeros_like(k1)
        qs = np.zeros_like(q1)
        for j in range(2):
            sl = slice(128 * j, 128 * (j + 1))
            ks[sl] = k1[sl] * crep + (R.T @ k1[sl]) * srep
            qs[sl] = q1[sl] * crep + (R.T @ q1[sl]) * srep

        gram = np.zeros((128, 256), np.float32)
        for j in range(2):
            gram[:, 128 * j:128 * (j + 1)] = (
                ks[128 * j:128 * (j + 1)] @ vT[128 * j:128 * (j + 1), 1:T + 1].T)
        ksum = k1.sum(axis=1)                      # [256]
        vsum = vT[:, 1:T + 1].sum(axis=1)
        cores.append(dict(d=d, q1=q1, qs=qs, oT=oT, vT=vT, gram=gram,
                          ksum=ksum, vsum=vsum))

    for pair in range(4):
        a, b2 = cores[2 * pair], cores[2 * pair + 1]
        gram = a["gram"] + b2["gram"]
        ksum = a["ksum"] + b2["ksum"]
        vsum = a["vsum"] + b2["vsum"]
        for cc in (a, b2):
            cc["gram_r"], cc["ksum_r"], cc["vsum_r"] = gram, ksum, vsum

    outs = []
    for c in range(NCORES):
        st = cores[c]
        d = st["d"]
        q1, qs, oT, vT = st["q1"], st["qs"], st["oT"], st["vT"]
        hmask = d["hmask"].astype(np.float32)
        wl6 = d["wl6"].astype(np.float32)
        wp = d["wp"].astype(np.float32)
        gram, ksum, vsum = st["gram_r"], st["ksum_r"], st["vsum_r"]

        res = np.zeros((256, T), np.float32)
        for j in range(2):
            sl = slice(128 * j, 128 * (j + 1))
            zsc = (SCALE / N) * ksum[sl]                     # [128]
            zblk = zsc[:, None] * hmask                      # [128,128]
            zps = zblk.T @ q1[sl]                            # [128, T]
            kvblk = np.zeros((128, 128), np.float32)
            for aa in range(4):
                s2 = slice(32 * aa, 32 * (aa + 1))
                kvblk[s2, s2] = KSC * gram[s2, 128 * j + 32 * aa:
                                           128 * j + 32 * (aa + 1)]
            aps = kvblk.T @ qs[sl]
            a1 = (1.0 / zps + 1.0) * aps
            lepe = (wl6[:, 3 * j:3 * j + 1] * vT[sl, 0:T]
                    + wl6[:, 3 * j + 1:3 * j + 2] * vT[sl, 1:T + 1]
                    + wl6[:, 3 * j + 2:3 * j + 3] * vT[sl, 2:T + 2])
            c1 = zps * (-vsum[sl] / N)[:, None] + lepe
            res[sl] = a1 + c1
        y = res * oT
        outs.append((y.T @ wp).astype(np.float32))

    full = np.zeros((B, N, 256), np.float32)
    for c in range(NCORES):
        b = c // 2
        t0 = (c % 2) * T
        full[b, t0:t0 + T] = outs[c]
    return full


if __name__ == "__main__" and os.environ.get("KERNEL_SELFTEST"):
    sys.path.insert(0, os.path.dirname(os.path.abspath(__file__)))
    import reference
    inputs = {k: np.asarray(v) for k, v in reference.setup_inputs().items()}
    expected = np.asarray(reference.reference(**inputs))
    per_core = _host_prep(**inputs)
    got = _numpy_pipeline(per_core)
    rel = np.linalg.norm(got - expected) / np.linalg.norm(expected)
    print("selftest rel err:", rel, "max abs:", np.abs(got - expected).max())

if __name__ == "__main__" and os.environ.get("KERNEL_BUILD"):
    nc = _build_nc()
    print("build OK")
